# revision 1
# baseline (speedup 1.0000x reference)
"""Trainium2 Bass kernel for nn_DepthPrediction (multi-view stereo depth).

Strategy (8 NeuronCores, SPMD single program):
  - Shard: core k handles batch b = k//4 and depth planes [8*(k%4), 8*(k%4)+8).
  - Per (b,d): homography warp of 2 src views via on-device fp16 "patch maps"
    (130x130 positions x [dy2,dx2,c16] fp16 chunks = 128B) gathered with one
    indirect-DMA descriptor per pixel; bilinear interp + cumulative cost
    (L2 over 16 ch) in pixel-major fp16 on DVE; 5x5 adaptive aggregation
    (depth-similarity softmax x feature-similarity weight); per-core softmax
    partials (max / sum exp / sum exp*depth) over the 8 local planes.
  - Host: trivial glue — 4x4 matrix algebra, shard/pack inputs, combine the
    4-way softmax partials per batch (log-sum-exp merge) into the output.

Self-contained: hardcodes all shapes from the problem spec.
"""

import math

import numpy as np

import concourse.bacc as bacc
import concourse.bass as bass
import concourse.mybir as mybir
import concourse.tile as tile
from concourse.bass_utils import run_bass_kernel_spmd
from concourse.tile_rust import add_dep_helper

F32 = mybir.dt.float32
F16 = mybir.dt.float16
I32 = mybir.dt.int32

AF = mybir.ActivationFunctionType
OP = mybir.AluOpType
AX = mybir.AxisListType

# problem shapes
V, B, C, H, W, D = 3, 2, 16, 128, 128, 32
NCORES = 8
DPC = D // (NCORES // B)  # depth planes per core = 8
NV = V - 1  # src views = 2

PW = W + 4          # padded map width (x0 clipped to [-2,129] -> 132)
POS = PW * PW       # patch positions
CH = 128            # chunk elems: [dy2,dx2,c16]+pad64 fp16 = 256B
BIG = 1024.0        # float floor-shift
KPOS = -(BIG - 2.0) * PW - (BIG - 2.0)  # pos = yiC*PW + xiC + KPOS
PADX = W + 4        # x-padded tiles for 5-tap aggregation
SCALE = W / (W - 1.0)  # grid_sample align_corners=False fold
ESHIFT = -9.0       # exp(|dnb-d|) stabilization shift (|delta| <= 9)

_cached = {}


def _ap(base, off, dims):
    """Raw AP on the same tensor as `base` (an AP), offset in elements."""
    return bass.AP(base.tensor, base.offset + off, dims)


def build_program(debug=False):
    nc = bacc.Bacc("TRN2", target_bir_lowering=False, debug=False,
                   num_devices=NCORES)

    refF = nc.dram_tensor("refF", [C, H, W], F32, kind="ExternalInput")
    srcF = nc.dram_tensor("srcF", [NV, C, H, W], F32, kind="ExternalInput")
    dep = nc.dram_tensor("dep", [DPC, H, W], F32, kind="ExternalInput")
    amap = nc.dram_tensor("amap", [NV, 3, H, W], F32, kind="ExternalInput")
    tvec = nc.dram_tensor("tvec", [128, 8], F32, kind="ExternalInput")
    ident = nc.dram_tensor("ident", [128, 128], F32, kind="ExternalInput")
    out3 = nc.dram_tensor("out3", [3, H, W], F32, kind="ExternalOutput")
    if debug:
        dbg_cost = nc.dram_tensor("dbg_cost", [DPC, H, W], F32,
                                  kind="ExternalOutput")
        dbg_agg = nc.dram_tensor("dbg_agg", [DPC, H, W], F32,
                                 kind="ExternalOutput")
        dbg_wf = nc.dram_tensor("dbg_wf", [25, H, W], F32,
                                kind="ExternalOutput")
        dbg_coord = nc.dram_tensor("dbg_coord", [NV, 3, H, W], F32,
                                   kind="ExternalOutput")
        dbg_cum = nc.dram_tensor("dbg_cum", [NV, H, W, C], F32,
                                 kind="ExternalOutput")
        dbg_pmap = nc.dram_tensor("dbg_pmap", [NV, POS, CH], F16,
                                  kind="ExternalOutput")
    # internal patch maps, one per src view
    pmap = [nc.dram_tensor(f"pmap{v}", [POS, CH], F16, kind="Internal")
            for v in range(NV)]

    with nc.allow_low_precision("fp16 pipeline by design"), \
            tile.TileContext(nc) as tc:
        ctx_pools = []

        def pool(name, bufs=1, **kw):
            p = tc.tile_pool(name=name, bufs=bufs, **kw)
            ctx_pools.append(p)
            return p.__enter__()

        pp = pool("persist", 1)     # long-lived tiles
        psp = pool("psum", 2, space="PSUM")
        prep = tc.tile_pool(name="prep", bufs=1)
        pr = prep.__enter__()

        # ---------------- constant / persistent loads ----------------
        amapL = pp.tile([128, NV * 3 * W], F32, tag="amapL")  # [y,(v,row,x)]
        nc.sync.dma_start(out=amapL[:], in_=_ap(
            amap.ap(), 0, [[W, 128], [3 * H * W, NV], [H * W, 3], [1, W]]))
        tvecT = pp.tile([128, 8], F32, tag="tvecT")
        nc.sync.dma_start(out=tvecT[:], in_=tvec.ap())
        identT = pp.tile([128, 128], F32, tag="identT")
        nc.sync.dma_start(out=identT[:], in_=ident.ap())

        def tv(col):  # [128,1] per-partition scalar AP
            return tvecT[:, col:col + 1]

        # ---------------- patch map build (per src view) ----------------
        build_dumps = []
        for v in range(NV):
            # TF16 [y, (c,x)] fp16  (cast during DMA, SWDGE)
            tf = pr.tile([128, C * W], F16, tag="tf")
            nc.gpsimd.dma_start(
                out=tf[:], in_=_ap(srcF.ap(), v * C * H * W,
                                   [[W, 128], [H * W, C], [1, W]]))
            # down-shifted copy: tfdn[y] = tf[y+1]
            tfdn = pr.tile([128, C * W], F16, tag="tfdn")
            nc.vector.memset(tfdn[:], 0.0)
            nc.sync.dma_start(out=tfdn[0:127, :], in_=tf[1:128, :])

            # staged [y, (xp130, dy2, dx2, c16)] fp16 ; row y -> pos row y+1
            staged = pr.tile([128, PW * CH], F16, tag="staged")
            nc.vector.memset(staged[:], 0.0)
            copy_fns = [nc.vector.tensor_copy,
                        lambda out, in_: nc.scalar.copy(out=out, in_=in_),
                        nc.gpsimd.tensor_copy,
                        nc.vector.tensor_copy]
            i = 0
            for dy, src in ((0, tf), (1, tfdn)):
                for dx in (0, 1):
                    # staged[y, xp, dy, dx, c] = src[y, c, xp-2+dx]
                    xp_lo = 2 - dx
                    n_xp = 128
                    src_ap = _ap(src[:], (xp_lo - 2 + dx),
                                 [[C * W, 128], [1, n_xp], [W, C]])
                    dst_ap = _ap(staged[:], xp_lo * CH + dy * 32 + dx * 16,
                                 [[PW * CH, 128], [CH, n_xp], [1, C]])
                    copy_fns[i](out=dst_ap, in_=src_ap)
                    i += 1
            # dump rows 2..129 of the pos grid (partition y -> row y+2)
            d1 = nc.sync.dma_start(
                out=_ap(pmap[v].ap(), 2 * PW * CH,
                        [[PW * CH, 128], [1, PW * CH]]),
                in_=staged[:])
            # zero border rows 0, 1, 130, 131
            zrow = pr.tile([2, PW * CH], F16, tag="zrow")
            nc.vector.memset(zrow[:], 0.0)
            d2 = nc.sync.dma_start(
                out=_ap(pmap[v].ap(), 0, [[PW * CH, 2], [1, PW * CH]]),
                in_=zrow[:])
            d3 = nc.sync.dma_start(
                out=_ap(pmap[v].ap(), 130 * PW * CH,
                        [[PW * CH, 2], [1, PW * CH]]),
                in_=zrow[:])
            dlist = [d1, d2, d3]
            # row r=1 (y0=-1): dy=1 planes hold F row 0
            row1 = pr.tile([1, PW * CH], F16, tag="row1")
            nc.vector.memset(row1[:], 0.0)
            for dx in (0, 1):
                xp_lo = 2 - dx
                nc.vector.tensor_copy(
                    out=_ap(row1[:], xp_lo * CH + 1 * 32 + dx * 16,
                            [[PW * CH, 1], [CH, 128], [1, C]]),
                    in_=_ap(tf[:], (xp_lo - 2 + dx),
                            [[C * W, 1], [1, 128], [W, C]]))
            d4 = nc.sync.dma_start(
                out=_ap(pmap[v].ap(), PW * CH, [[PW * CH, 1], [1, PW * CH]]),
                in_=row1[:])
            dlist.append(d4)
            build_dumps.append(tuple(dlist))
            if debug:
                dd = nc.sync.dma_start(
                    out=_ap(dbg_pmap.ap(), v * POS * CH,
                            [[POS * CH, 1], [1, POS * CH]]),
                    in_=_ap(pmap[v].ap(), 0, [[POS * CH, 1], [1, POS * CH]]))
                for dx_ in dlist:
                    add_dep_helper(dd.ins, dx_.ins, reason="dump after build")

        # ---------------- ref prep ----------------
        tr = pr.tile([128, C * W], F32, tag="tr")  # [y,(c,x)] f32
        nc.sync.dma_start(out=tr[:], in_=_ap(
            refF.ap(), 0, [[W, 128], [H * W, C], [1, W]]))
        # refC [y,(x,c)] fp16
        refC = pp.tile([128, W * C], F16, tag="refC")
        nc.vector.tensor_copy(
            out=_ap(refC[:], 0, [[W * C, 128], [C, W], [1, C]]),
            in_=_ap(tr[:], 0, [[C * W, 128], [1, W], [W, C]]))
        # refPadC [y,(xp132,c)] fp16, x' = x+2
        refPadC = pp.tile([128, PADX * C], F16, tag="refPadC")
        nc.vector.memset(refPadC[:], 0.0)
        nc.scalar.copy(out=refPadC[:, 2 * C:(2 + W) * C], in_=refC[:])
        # 4 partition-shifted copies (ty in {0,1,3,4}; center ty=2 = refPadC)
        refSC = {}
        for ty in (0, 1, 3, 4):
            t = pp.tile([128, PADX * C], F16, tag=f"refSC{ty}")
            k = ty - 2
            nc.vector.memset(t[:], 0.0)
            if k < 0:
                nc.sync.dma_start(out=t[-k:128, :], in_=refPadC[0:128 + k, :])
            else:
                nc.sync.dma_start(out=t[0:128 - k, :], in_=refPadC[k:128, :])
            refSC[ty] = t
        refSC[2] = refPadC

        # R2 = sum_c ref^2  [y, x] fp16 (+ padded & shifted)
        sqt = pr.tile([128, W * C], F16, tag="sqt")
        nc.vector.tensor_tensor(out=sqt[:], in0=refC[:], in1=refC[:],
                                op=OP.mult)
        r2 = pp.tile([128, W], F16, tag="r2")
        nc.vector.tensor_reduce(
            out=r2[:], in_=_ap(sqt[:], 0, [[W * C, 128], [C, W], [1, C]]),
            axis=AX.X, op=OP.add)
        r2p = pp.tile([128, PADX], F16, tag="r2p")
        nc.vector.memset(r2p[:], 0.0)
        nc.scalar.copy(out=r2p[:, 2:2 + W], in_=r2[:])
        r2SC = {}
        for ty in (0, 1, 3, 4):
            t = pp.tile([128, PADX], F16, tag=f"r2SC{ty}")
            k = ty - 2
            nc.vector.memset(t[:], 0.0)
            if k < 0:
                nc.sync.dma_start(out=t[-k:128, :], in_=r2p[0:128 + k, :])
            else:
                nc.sync.dma_start(out=t[0:128 - k, :], in_=r2p[k:128, :])
            r2SC[ty] = t
        r2SC[2] = r2p

        prep.__exit__(None, None, None)
        wp = pool("work", 2)        # small per-(b,d) working tiles
        bp = pool("big4", 4)        # shared 4KB scratch (tag s4k)
        cp = pool("cumdif", 2)      # cum/diff accumulators
        gp = pool("gath", 2)        # gather destinations

        # ---------------- w_feat volume [y, (x, t25)] fp16 ----------------
        wfvol = pp.tile([128, W * 25], F16, tag="wfvol")
        wf_engines = [nc.vector, nc.gpsimd]
        for ty in range(5):
            for tx in range(5):
                t = ty * 5 + tx
                wf_ap = _ap(wfvol[:], t, [[W * 25, 128], [25, W], [1, 1]])
                if t == 12:
                    nc.vector.memset(wf_ap, 0.0)
                    continue
                # X = sum_c ref[y,x,c] * refSC[ty][y, x+tx, c]
                xprod = bp.tile([128, W * C], F16, tag="s4k")
                eng = wf_engines[(t // 2) % 2]
                eng.tensor_tensor(
                    out=xprod[:],
                    in0=_ap(refSC[ty][:], tx * C,
                            [[PADX * C, 128], [C, W], [1, C]]),
                    in1=refC[:], op=OP.mult)
                xd = wp.tile([128, W], F16, tag="xd")
                nc.vector.tensor_reduce(
                    out=xd[:],
                    in_=_ap(xprod[:], 0, [[W * C, 128], [C, W], [1, C]]),
                    axis=AX.X, op=OP.add)
                # wf2 = R2SC[ty][x+tx] + R2 - 2X   (>= 0)
                wf2 = wp.tile([128, W], F16, tag="wf2")
                nc.vector.scalar_tensor_tensor(
                    out=wf2[:], in0=xd[:], scalar=-2.0, in1=r2[:],
                    op0=OP.mult, op1=OP.add)
                wf2b = wp.tile([128, W], F16, tag="wf2b")
                nc.vector.tensor_tensor(
                    out=wf2b[:], in0=wf2[:],
                    in1=_ap(r2SC[ty][:], tx, [[PADX, 128], [1, W]]),
                    op=OP.add)
                # clamp tiny negatives from fp16 rounding, then sqrt
                wf2c = wp.tile([128, W], F16, tag="wf2c")
                nc.vector.tensor_scalar(
                    out=wf2c[:], in0=wf2b[:], scalar1=0.0, scalar2=None,
                    op0=OP.max)
                nc.scalar.activation(out=wf_ap, in_=wf2c[:], func=AF.Sqrt)
        if debug:
            wfdbg = pp.tile([128, W], F32, tag="wfdbg")
            for t in range(25):
                nc.scalar.copy(out=wfdbg[:],
                               in_=_ap(wfvol[:], t,
                                       [[W * 25, 128], [25, W], [1, 1]]))
                nc.sync.dma_start(out=dbg_wf.ap()[t].rearrange("y x -> y x"),
                                  in_=wfdbg[:])

        # depth planes f32 [y,(x,d)] and agg store
        depD = pp.tile([128, W * DPC], F32, tag="depD")
        aggT = pp.tile([128, W * DPC], F32, tag="aggT")

        # ---------------- per depth-plane pipeline ----------------
        for di in range(DPC):
            depf = wp.tile([128, W], F32, tag="depf")
            nc.sync.dma_start(out=depf[:], in_=dep.ap()[di])
            nc.vector.tensor_copy(
                out=_ap(depD[:], di, [[W * DPC, 128], [DPC, W], [1, 1]]),
                in_=depf[:])
            # fp16 padded depth + 4 shifts
            depp = wp.tile([128, PADX], F16, tag="depp")
            nc.vector.memset(depp[:], 0.0)
            nc.scalar.copy(out=depp[:, 2:2 + W], in_=depf[:])
            depSC = {}
            for ty in (0, 1, 3, 4):
                t = wp.tile([128, PADX], F16, tag=f"depSC{ty}")
                k = ty - 2
                nc.vector.memset(t[:], 0.0)
                if k < 0:
                    nc.sync.dma_start(out=t[-k:128, :],
                                      in_=depp[0:128 + k, :])
                else:
                    nc.sync.dma_start(out=t[0:128 - k, :],
                                      in_=depp[k:128, :])
                depSC[ty] = t
            depSC[2] = depp

            cum = cp.tile([128, W * C], F16, tag="cum")
            diff = cp.tile([128, W * C], F16, tag="diff")
            csq = {}
            for v in range(NV):
                # ---- coordinates (pixel-major [y,x] f32) ----
                def arow(r):
                    return _ap(amapL[:], (v * 3 + r) * W,
                               [[NV * 3 * W, 128], [1, W]])
                mx = wp.tile([128, W], F32, tag="mx")
                my = wp.tile([128, W], F32, tag="my")
                dn = wp.tile([128, W], F32, tag="dn")
                nc.vector.tensor_tensor(out=mx[:], in0=arow(0), in1=depf[:],
                                        op=OP.mult)
                nc.vector.tensor_tensor(out=my[:], in0=arow(1), in1=depf[:],
                                        op=OP.mult)
                nc.vector.tensor_tensor(out=dn[:], in0=arow(2), in1=depf[:],
                                        op=OP.mult)
                nx = wp.tile([128, W], F32, tag="nx")
                ny = wp.tile([128, W], F32, tag="ny")
                dnt = wp.tile([128, W], F32, tag="dnt")
                nc.vector.tensor_scalar(out=nx[:], in0=mx[:],
                                        scalar1=tv(v * 3 + 0), scalar2=None,
                                        op0=OP.add)
                nc.vector.tensor_scalar(out=ny[:], in0=my[:],
                                        scalar1=tv(v * 3 + 1), scalar2=None,
                                        op0=OP.add)
                nc.vector.tensor_scalar(out=dnt[:], in0=dn[:],
                                        scalar1=tv(v * 3 + 2), scalar2=None,
                                        op0=OP.add)
                rec = wp.tile([128, W], F32, tag="rec")
                nc.vector.reciprocal(out=rec[:], in_=dnt[:])
                gxB = wp.tile([128, W], F32, tag="gxB")
                gyB = wp.tile([128, W], F32, tag="gyB")
                nc.vector.tensor_tensor(out=gxB[:], in0=nx[:], in1=rec[:],
                                        op=OP.mult)
                nc.vector.tensor_tensor(out=gyB[:], in0=ny[:], in1=rec[:],
                                        op=OP.mult)
                # += BIG - 0.5 (pixel-center shift folded: tvec already holds
                # t*SCALE; the -0.5 of grid_sample plus BIG)
                nc.scalar.activation(out=gxB[:], in_=gxB[:], func=AF.Identity,
                                     bias=tv(6))
                nc.scalar.activation(out=gyB[:], in_=gyB[:], func=AF.Identity,
                                     bias=tv(6))

                def floorfrac(g, nm):
                    xi0 = wp.tile([128, W], I32, tag=f"i{nm}")
                    nc.vector.tensor_copy(out=xi0[:], in_=g[:])
                    xf = wp.tile([128, W], F32, tag=f"xf{nm}")
                    nc.scalar.copy(out=xf[:], in_=xi0[:])
                    gt = wp.tile([128, W], F32, tag=f"gt{nm}")
                    nc.vector.tensor_tensor(out=gt[:], in0=xf[:], in1=g[:],
                                            op=OP.is_gt)
                    xif = wp.tile([128, W], F32, tag=f"xif{nm}")
                    nc.vector.tensor_tensor(out=xif[:], in0=xf[:], in1=gt[:],
                                            op=OP.subtract)
                    fr = wp.tile([128, W], F32, tag=f"fr{nm}")
                    nc.vector.tensor_tensor(out=fr[:], in0=g[:], in1=xif[:],
                                            op=OP.subtract)
                    return xif, fr

                xif, fx = floorfrac(gxB, "x")
                yif, fy = floorfrac(gyB, "y")
                xiC = wp.tile([128, W], F32, tag="xiC")
                yiC = wp.tile([128, W], F32, tag="yiC")
                nc.vector.tensor_scalar(out=xiC[:], in0=xif[:],
                                        scalar1=BIG + 129.0,
                                        scalar2=BIG - 2.0,
                                        op0=OP.min, op1=OP.max)
                nc.vector.tensor_scalar(out=yiC[:], in0=yif[:],
                                        scalar1=BIG + 129.0,
                                        scalar2=BIG - 2.0,
                                        op0=OP.min, op1=OP.max)
                posF = wp.tile([128, W], F32, tag="posF")
                nc.vector.scalar_tensor_tensor(
                    out=posF[:], in0=yiC[:], scalar=float(PW), in1=xiC[:],
                    op0=OP.mult, op1=OP.add)
                posK = wp.tile([128, W], F32, tag="posK")
                nc.vector.tensor_scalar(out=posK[:], in0=posF[:],
                                        scalar1=float(KPOS), scalar2=None,
                                        op0=OP.add)

                # fp16 fractional weights, pair-duplicated [y,(x,2)]
                fx16 = wp.tile([128, W], F16, tag="fx16")
                fy16 = wp.tile([128, W], F16, tag="fy16")
                fxm16 = wp.tile([128, W], F16, tag="fxm16")
                fym16 = wp.tile([128, W], F16, tag="fym16")
                nc.scalar.copy(out=fx16[:], in_=fx[:])
                nc.scalar.copy(out=fy16[:], in_=fy[:])
                nc.vector.tensor_scalar(out=fxm16[:], in0=fx[:], scalar1=-1.0,
                                        scalar2=1.0, op0=OP.mult, op1=OP.add)
                nc.vector.tensor_scalar(out=fym16[:], in0=fy[:], scalar1=-1.0,
                                        scalar2=1.0, op0=OP.mult, op1=OP.add)
                wd = {}
                for (tnm, fa, fb) in (("00", fxm16, fym16),
                                      ("01", fx16, fym16),
                                      ("10", fxm16, fy16),
                                      ("11", fx16, fy16)):
                    wt = wp.tile([128, W * 2], F16, tag=f"wd{tnm}")
                    nc.vector.tensor_tensor(
                        out=_ap(wt[:], 0, [[W * 2, 128], [2, W], [1, 2]]),
                        in0=_ap(fa[:], 0, [[W, 128], [1, W], [0, 2]]),
                        in1=_ap(fb[:], 0, [[W, 128], [1, W], [0, 2]]),
                        op=OP.mult)
                    wd[tnm] = wt

                # ---- gather (dma_gather, wrapped-16 idx layout) ----
                # fold posF [x-part, y] -> wrapped16 [16, y*8+xh] via PE
                wr16 = wp.tile([128, W * 8], mybir.dt.int16, tag="wr16")
                for xh in range(8):
                    ps = psp.tile([128, W], F32, tag="psfold")
                    nc.tensor.matmul(out=ps[0:16, :],
                                     lhsT=identT[:, 16 * xh:16 * xh + 16],
                                     rhs=posK[:], start=True, stop=True)
                    nc.vector.tensor_copy(
                        out=_ap(wr16[:], xh, [[W * 8, 16], [8, W]]),
                        in_=ps[0:16, :])
                # replicate partitions [0:16) -> all 128 (7 window copies)
                wrep = wp.tile([128, W * 8], mybir.dt.int16, tag="wrep")
                nc.sync.dma_start(out=wrep[0:16, :], in_=wr16[0:16, :])
                for g in range(1, 8):
                    nc.sync.dma_start(out=wrep[16 * g:16 * g + 16, :],
                                      in_=wr16[0:16, :])
                G = gp.tile([128, W * CH], F16, tag="G")
                for t in range(4):
                    gi = nc.gpsimd.dma_gather(
                        out_ap=_ap(G[:], t * 32 * CH,
                                   [[W * CH, 128], [CH, 32], [1, CH]]),
                        in_ap=pmap[v].ap(),
                        idxs_ap=_ap(wrep[:], t * 32 * 8,
                                    [[W * 8, 128], [1, 32 * 8]]),
                        num_idxs=32 * 128, num_idxs_reg=32 * 128,
                        elem_size=CH, queue_num=0)
                    for dma_i in build_dumps[v]:
                        add_dep_helper(gi.ins, dma_i.ins,
                                       reason="patch map build before gather")

                # ---- bilinear taps: acc = sum_t w_t * G_t  [y,(x,c)] ----
                def tap(dy, dx):
                    return _ap(G[:], (dy * 2 + dx) * 16,
                               [[W * CH, 128], [CH, W], [2, 8], [1, 2]])

                def wap(tnm):
                    return _ap(wd[tnm][:], 0,
                               [[W * 2, 128], [2, W], [0, 8], [1, 2]])

                cview = [[2048, 128], [16, W], [2, 8], [1, 2]]
                acc = cum if v == 0 else bp.tile([128, W * C], F16, tag="s4k")
                p0 = bp.tile([128, W * C], F16, tag="s4k")
                p1 = bp.tile([128, W * C], F16, tag="s4k")
                nc.vector.tensor_tensor(out=_ap(acc[:], 0, cview),
                                        in0=tap(0, 0), in1=wap("00"),
                                        op=OP.mult)
                nc.vector.tensor_tensor(out=_ap(p0[:], 0, cview),
                                        in0=tap(0, 1), in1=wap("01"),
                                        op=OP.mult)
                nc.vector.tensor_tensor(out=_ap(p1[:], 0, cview),
                                        in0=tap(1, 0), in1=wap("10"),
                                        op=OP.mult)
                nc.vector.tensor_tensor(out=acc[:], in0=acc[:], in1=p0[:],
                                        op=OP.add)
                nc.vector.tensor_tensor(out=_ap(p0[:], 0, cview),
                                        in0=tap(1, 1), in1=wap("11"),
                                        op=OP.mult)
                nc.vector.tensor_tensor(out=p1[:], in0=p1[:], in1=p0[:],
                                        op=OP.add)
                nc.vector.tensor_tensor(out=acc[:], in0=acc[:], in1=p1[:],
                                        op=OP.add)

                # ---- cost_v = sum_c (ref - cum)^2 ----
                if v == 0:
                    nc.vector.tensor_tensor(out=diff[:], in0=refC[:],
                                            in1=cum[:], op=OP.subtract)
                else:
                    nc.vector.tensor_tensor(out=diff[:], in0=diff[:],
                                            in1=acc[:], op=OP.subtract)
                sq = bp.tile([128, W * C], F16, tag="s4k")
                nc.vector.tensor_tensor(out=sq[:], in0=diff[:], in1=diff[:],
                                        op=OP.mult)
                cs = wp.tile([128, W], F16, tag=f"csq{v}")
                nc.vector.tensor_reduce(
                    out=cs[:], in_=_ap(sq[:], 0, [[W * C, 128], [C, W], [1, C]]),
                    axis=AX.X, op=OP.add)
                csq[v] = cs
                if debug and di == 0:
                    dc = wp.tile([128, W], F32, tag="dcoord")
                    nc.vector.tensor_copy(out=dc[:], in_=posF[:])
                    nc.sync.dma_start(out=dbg_coord.ap()[v, 0], in_=dc[:])
                    dc2 = wp.tile([128, W], F32, tag="dcoord2")
                    nc.vector.tensor_copy(out=dc2[:], in_=fx[:])
                    nc.sync.dma_start(out=dbg_coord.ap()[v, 1], in_=dc2[:])
                    dc3 = wp.tile([128, W], F32, tag="dcoord3")
                    nc.vector.tensor_copy(out=dc3[:], in_=fy[:])
                    nc.sync.dma_start(out=dbg_coord.ap()[v, 2], in_=dc3[:])
                    du = wp.tile([128, W * C], F32, tag="dcum")
                    nc.vector.tensor_copy(out=du[:],
                                          in_=acc[:] if v else cum[:])
                    nc.sync.dma_start(
                        out=_ap(dbg_cum.ap(), v * H * W * C,
                                [[W * C, 128], [1, W * C]]),
                        in_=du[:])

            # cost = sqrt(min(c1sq, c2sq)), padded [y, 132] fp16
            cmin = wp.tile([128, W], F16, tag="cmin")
            nc.vector.tensor_tensor(out=cmin[:], in0=csq[0][:], in1=csq[1][:],
                                    op=OP.min)
            costp = wp.tile([128, PADX], F16, tag="costp")
            nc.vector.memset(costp[:], 0.0)
            nc.scalar.activation(out=costp[:, 2:2 + W], in_=cmin[:],
                                 func=AF.Sqrt)
            if debug:
                cdbg = wp.tile([128, W], F32, tag="cdbg")
                nc.scalar.copy(out=cdbg[:], in_=costp[:, 2:2 + W])
                nc.sync.dma_start(out=dbg_cost.ap()[di], in_=cdbg[:])
            costSC = {}
            for ty in (0, 1, 3, 4):
                t = wp.tile([128, PADX], F16, tag=f"costSC{ty}")
                k = ty - 2
                nc.vector.memset(t[:], 0.0)
                if k < 0:
                    nc.sync.dma_start(out=t[-k:128, :],
                                      in_=costp[0:128 + k, :])
                else:
                    nc.sync.dma_start(out=t[0:128 - k, :],
                                      in_=costp[k:128, :])
                costSC[ty] = t
            costSC[2] = costp

            # ---- aggregation: num/den over 25 taps ----
            num = wp.tile([128, W], F32, tag="num")
            den = wp.tile([128, W], F32, tag="den")
            for ty in range(5):
                # dvol[y, x, tx5] = depSC[ty][y, x+tx] - dep[y, x]
                dvol = wp.tile([128, W * 5], F16, tag="dvol")
                vv = [[W * 5, 128], [5, W], [1, 5]]
                nc.vector.tensor_tensor(
                    out=_ap(dvol[:], 0, vv),
                    in0=_ap(depSC[ty][:], 0, [[PADX, 128], [1, W], [1, 5]]),
                    in1=_ap(depp[:], 2, [[PADX, 128], [1, W], [0, 5]]),
                    op=OP.subtract)
                # exp(|.| - 9)
                nc.scalar.activation(out=dvol[:], in_=dvol[:], func=AF.Abs)
                evol = wp.tile([128, W * 5], F16, tag="evol")
                nc.scalar.activation(out=evol[:], in_=dvol[:], func=AF.Exp,
                                     bias=tv(7))
                # u = e * wf ; n = u * cnb
                uvol = wp.tile([128, W * 5], F16, tag="uvol")
                nc.vector.tensor_tensor(
                    out=_ap(uvol[:], 0, vv), in0=_ap(evol[:], 0, vv),
                    in1=_ap(wfvol[:], ty * 5, [[W * 25, 128], [25, W], [1, 5]]),
                    op=OP.mult)
                nvol = wp.tile([128, W * 5], F16, tag="nvol")
                nc.vector.tensor_tensor(
                    out=_ap(nvol[:], 0, vv), in0=_ap(uvol[:], 0, vv),
                    in1=_ap(costSC[ty][:], 0, [[PADX, 128], [1, W], [1, 5]]),
                    op=OP.mult)
                sty = wp.tile([128, W], F16, tag="sty")
                nc.vector.tensor_reduce(out=sty[:], in_=_ap(nvol[:], 0, vv),
                                        axis=AX.X, op=OP.add)
                ety = wp.tile([128, W], F16, tag="ety")
                nc.vector.tensor_reduce(out=ety[:], in_=_ap(evol[:], 0, vv),
                                        axis=AX.X, op=OP.add)
                if ty == 0:
                    nc.vector.tensor_copy(out=num[:], in_=sty[:])
                    nc.vector.tensor_copy(out=den[:], in_=ety[:])
                else:
                    nc.vector.tensor_tensor(out=num[:], in0=num[:],
                                            in1=sty[:], op=OP.add)
                    nc.vector.tensor_tensor(out=den[:], in0=den[:],
                                            in1=ety[:], op=OP.add)
            rden = wp.tile([128, W], F32, tag="rden")
            nc.vector.reciprocal(out=rden[:], in_=den[:])
            agg_ap = _ap(aggT[:], di, [[W * DPC, 128], [DPC, W], [1, 1]])
            nc.vector.tensor_tensor(out=agg_ap, in0=num[:], in1=rden[:],
                                    op=OP.mult)
            if debug:
                adbg = wp.tile([128, W], F32, tag="adbg")
                nc.vector.tensor_tensor(out=adbg[:], in0=num[:], in1=rden[:],
                                        op=OP.mult)
                nc.sync.dma_start(out=dbg_agg.ap()[di], in_=adbg[:])

        # ---------------- per-core softmax partials ----------------
        def aggap(di):
            return _ap(aggT[:], di, [[W * DPC, 128], [DPC, W], [1, 1]])

        def depap(di):
            return _ap(depD[:], di, [[W * DPC, 128], [DPC, W], [1, 1]])

        m = pp.tile([128, W], F32, tag="m")
        nc.vector.tensor_tensor(out=m[:], in0=aggap(0), in1=aggap(1),
                                op=OP.max)
        for di in range(2, DPC):
            nc.vector.tensor_tensor(out=m[:], in0=m[:], in1=aggap(di),
                                    op=OP.max)
        s0 = pp.tile([128, W], F32, tag="s0")
        s1 = pp.tile([128, W], F32, tag="s1")
        for di in range(DPC):
            t = wp.tile([128, W], F32, tag="et")
            nc.vector.tensor_tensor(out=t[:], in0=aggap(di), in1=m[:],
                                    op=OP.subtract)
            e = wp.tile([128, W], F32, tag="ee")
            nc.scalar.activation(out=e[:], in_=t[:], func=AF.Exp)
            t1 = wp.tile([128, W], F32, tag="t1")
            nc.vector.tensor_tensor(out=t1[:], in0=e[:], in1=depap(di),
                                    op=OP.mult)
            if di == 0:
                nc.vector.tensor_copy(out=s0[:], in_=e[:])
                nc.vector.tensor_copy(out=s1[:], in_=t1[:])
            else:
                nc.vector.tensor_tensor(out=s0[:], in0=s0[:], in1=e[:],
                                        op=OP.add)
                nc.vector.tensor_tensor(out=s1[:], in0=s1[:], in1=t1[:],
                                        op=OP.add)
        nc.sync.dma_start(out=out3.ap()[0], in_=m[:])
        nc.sync.dma_start(out=out3.ap()[1], in_=s0[:])
        nc.sync.dma_start(out=out3.ap()[2], in_=s1[:])

        for p in reversed(ctx_pools):
            p.__exit__(None, None, None)

    nc.compile()
    return nc


def host_prep(features, intrinsics, cam_to_world, depth_hypo):
    """Build the 8 per-core input maps. All O(small) except slicing."""
    features = np.asarray(features, dtype=np.float32)
    intrinsics = np.asarray(intrinsics, dtype=np.float32)
    cam_to_world = np.asarray(cam_to_world, dtype=np.float32)
    depth_hypo = np.asarray(depth_hypo, dtype=np.float32)

    ys, xs = np.meshgrid(np.arange(H, dtype=np.float32),
                         np.arange(W, dtype=np.float32), indexing="ij")
    in_maps = []
    for k in range(NCORES):
        b = k // (NCORES // B)
        dlo = DPC * (k % (NCORES // B))
        amap = np.zeros((NV, 3, H, W), np.float32)
        tv = np.zeros((8,), np.float32)
        for vi in range(1, V):
            src_w2c = np.linalg.inv(cam_to_world[vi, b])
            ref_w2c = np.linalg.inv(cam_to_world[0, b])
            src_KK = src_w2c.copy()
            src_KK[:3, :3] = intrinsics[vi, b]
            ref_KK = ref_w2c.copy()
            ref_KK[:3, :3] = intrinsics[0, b]
            proj = (src_KK @ src_w2c) @ np.linalg.inv(ref_KK @ ref_w2c)
            rot, trans = proj[:3, :3], proj[:3, 3]
            A = (rot[:, 0:1, None] * xs[None] + rot[:, 1:2, None] * ys[None]
                 + rot[:, 2:3, None])  # [3, H, W]
            v = vi - 1
            amap[v, 0] = A[0] * SCALE
            amap[v, 1] = A[1] * SCALE
            amap[v, 2] = A[2]
            tv[v * 3 + 0] = trans[0] * SCALE
            tv[v * 3 + 1] = trans[1] * SCALE
            tv[v * 3 + 2] = trans[2]
        tv[6] = BIG - 0.5
        tv[7] = ESHIFT
        in_maps.append({
            "refF": np.ascontiguousarray(features[0, b].transpose(0, 2, 1)),
            "srcF": np.ascontiguousarray(features[1:, b]),
            "dep": np.ascontiguousarray(
                depth_hypo[b, dlo:dlo + DPC].transpose(0, 2, 1)),
            "amap": np.ascontiguousarray(amap.transpose(0, 1, 3, 2)),
            "tvec": np.tile(tv[None, :], (128, 1)).astype(np.float32),
            "ident": np.eye(128, dtype=np.float32),
        })
    return in_maps


def host_combine(results):
    """Merge per-core softmax partials (m, s0, s1) into [B, H, W]."""
    out = np.zeros((B, H, W), np.float32)
    per_b = NCORES // B
    for b in range(B):
        parts = [np.asarray(results[b * per_b + j]["out3"]) for j in range(per_b)]
        parts = [p.transpose(0, 2, 1) for p in parts]  # [3, x, y] -> [3, y, x]
        ms = np.stack([p[0] for p in parts])         # [4, H, W]
        M = ms.max(axis=0)
        S0 = np.zeros((H, W), np.float64)
        S1 = np.zeros((H, W), np.float64)
        for p in parts:
            w = np.exp(p[0] - M)
            S0 += w * p[1]
            S1 += w * p[2]
        out[b] = (S1 / S0).astype(np.float32)
    return out


def _run_sim(nc, in_maps):
    from concourse.bass_interp import CoreSim
    results = []
    for core in range(NCORES):
        sim = CoreSim(nc, trace=False)
        for k, v in in_maps[core].items():
            sim.tensor(k)[:] = v
        sim.simulate()
        results.append({"out3": np.array(sim.tensor("out3"))})
    return results


def kernel(**inputs):
    if "nc" not in _cached:
        _cached["nc"] = build_program()
    nc = _cached["nc"]
    in_maps = host_prep(**inputs)
    if _cached.get("hw_broken"):
        return host_combine(_run_sim(nc, in_maps))
    try:
        res = run_bass_kernel_spmd(nc, in_maps, core_ids=list(range(NCORES)))
        return host_combine(res.results)
    except Exception:
        _cached["hw_broken"] = True
        return host_combine(_run_sim(nc, in_maps))


if __name__ == "__main__":
    import reference
    inp = reference.setup_inputs()
    inp = {k: np.asarray(v) for k, v in inp.items()}
    out = kernel(**inp)
    print("kernel out", out.shape, out.dtype)



# revision 2
# speedup vs baseline: 16.7243x; 16.7243x over previous
"""Trainium2 Bass kernel for nn_DepthPrediction — ap_gather version.

Strategy (8 NeuronCores, SPMD):
  - Shard: core k handles batch b = k//4 and depth planes [8*(k%4), +8).
  - Warp via gpsimd ap_gather (SBUF->SBUF ucode gather; the SWDGE
    dma_gather path aborts on this runtime).  Layout: 128 partitions =
    8 blocks x 16 channels; block k owns image rows [16k, 16k+16); free
    axis = 2048 pixels in (x, y_local) order, which makes the ap_gather
    index wrap coincide with the plain [y, x] pixel layout.
  - Table: host-built padded pair table per view: tab[pos] =
    (Fpad[pos], Fpad[pos+1]), pos = r*132 + c, r in [0,133), c in
    [0,132); r = y0+2, c = x0+2, zero border.  One gather (d=2) per
    bilinear row pair: idx -> (v00, v01), idx+132 -> (v10, v11).
  - One view table resident at a time; per-depth diff (ref - cum)
    spills to DRAM between the two view phases.
  - Cost: PE matmul with a block-selector matrix reduces over the 16
    channel partitions; SBUF->SBUF DMA reshapes [8, 2048] -> [y, x].
  - Aggregation (5x5 adaptive) + per-core softmax partials in pixel
    layout, as in the dma_gather version.
  - Host: 4x4 matrix algebra, table/shard packing, log-sum-exp merge
    of the 4-way softmax partials per batch.

Self-contained: hardcodes all shapes from the problem spec.
"""

import numpy as np

import concourse.bacc as bacc
import concourse.bass as bass
import concourse.mybir as mybir
import concourse.tile as tile
from concourse.bass_utils import run_bass_kernel_spmd
from concourse.tile_rust import add_dep_helper

F32 = mybir.dt.float32
F16 = mybir.dt.float16
I16 = mybir.dt.int16

AF = mybir.ActivationFunctionType
OP = mybir.AluOpType
AX = mybir.AxisListType

# problem shapes
V, B, C, H, W, D = 3, 2, 16, 128, 128, 32
NCORES = 8
DPC = D // (NCORES // B)  # depth planes per core = 8
NV = V - 1                # src views = 2

PW = 132                  # padded cols: x0 in [-2, 129] -> c = x0+2
PH = 133                  # padded rows: y0 in [-2, 129] -> r = y0+2; +1 row
NE = PH * PW              # 17556 table positions
NPIX = 16 * W             # pixels per partition block = 2048
PADX = W + 4              # x-padded pixel tiles for 5-tap aggregation
BIG = 1024.0              # float floor-shift
KPOS = -(BIG - 2.0) * PW - (BIG - 2.0)
SCALE = W / (W - 1.0)     # grid_sample align_corners=False fold
ESHIFT = -9.0             # exp(|dnb-d|) stabilization shift

_cached = {}


def _ap(base, off, dims):
    """Raw AP on the same tensor as `base` (an AP), offset in elements."""
    return bass.AP(base.tensor, base.offset + off, dims)


def build_program(debug=False):
    nc = bacc.Bacc("TRN2", target_bir_lowering=False, debug=False,
                   num_devices=NCORES)

    ptab = nc.dram_tensor("ptab", [NV, C * NE * 2], F16,
                          kind="ExternalInput")
    refF = nc.dram_tensor("refF", [C, H, W], F16, kind="ExternalInput")
    dep = nc.dram_tensor("dep", [DPC, H, W], F32, kind="ExternalInput")
    amap = nc.dram_tensor("amap", [NV, 3, H, W], F32, kind="ExternalInput")
    tvec = nc.dram_tensor("tvec", [128, 8], F32, kind="ExternalInput")
    bsel = nc.dram_tensor("bsel", [128, 8], F16, kind="ExternalInput")
    refBin = nc.dram_tensor("refBin", [128, NPIX], F16, kind="ExternalInput")
    ident = nc.dram_tensor("ident", [128, 128], F16, kind="ExternalInput")
    out3 = nc.dram_tensor("out3", [3, H, W], F32, kind="ExternalOutput")
    # DRAM scratch
    fscr = nc.dram_tensor("fscr", [2, H * W], F16, kind="Internal")
    cscr = nc.dram_tensor("cscr", [2, H * W], F16, kind="Internal")
    dscr = nc.dram_tensor("dscr", [DPC, H * W * C], F16, kind="Internal")
    if debug:
        dbg_warp = nc.dram_tensor("dbg_warp", [NV, 128, NPIX], F16,
                                  kind="ExternalOutput")
        dbg_cost = nc.dram_tensor("dbg_cost", [DPC, H, W], F32,
                                  kind="ExternalOutput")
        dbg_agg = nc.dram_tensor("dbg_agg", [DPC, H, W], F32,
                                 kind="ExternalOutput")
        dbg_coord = nc.dram_tensor("dbg_coord", [4, H, W], F32,
                                   kind="ExternalOutput")

    with nc.allow_low_precision("fp16 pipeline by design"), \
            tile.TileContext(nc) as tc:
        ctx_pools = []

        def pool(name, bufs=1, **kw):
            p = tc.tile_pool(name=name, bufs=bufs, **kw)
            ctx_pools.append(p)
            return p.__enter__()

        pp = pool("persist", 1)
        psp = pool("psum", 2, space="PSUM")

        # ---------------- persistent loads ----------------
        amapL = pp.tile([128, NV * 3 * W], F32, tag="amapL")  # [y,(v,row,x)]
        nc.sync.dma_start(out=amapL[:], in_=_ap(
            amap.ap(), 0, [[W, 128], [3 * H * W, NV], [H * W, 3], [1, W]]))
        tvecT = pp.tile([128, 8], F32, tag="tvecT")
        nc.sync.dma_start(out=tvecT[:], in_=tvec.ap())
        bselT = pp.tile([128, 8], F16, tag="bselT")
        nc.sync.dma_start(out=bselT[:], in_=bsel.ap())
        # ref features in block layout [p=(k,c), i=(x,y_l)] (host-permuted)
        refB = pp.tile([128, NPIX], F16, tag="refB")
        nc.sync.dma_start(out=refB[:], in_=refBin.ap())
        identT = pp.tile([128, 128], F16, tag="identT")
        nc.sync.dma_start(out=identT[:], in_=ident.ap())
        # view table (one view at a time)
        tabT = pp.tile([128, NE * 2], F16, tag="tabT")

        def tv(col):  # [128,1] per-partition scalar AP
            return tvecT[:, col:col + 1]

        # ---------------- pixel-layout ref prep (w_feat volume) ----------
        prep = tc.tile_pool(name="prep", bufs=1)
        pr = prep.__enter__()
        tr = pr.tile([128, C * W], F16, tag="tr")  # [y,(c,x)]
        nc.sync.dma_start(out=tr[:], in_=_ap(
            refF.ap(), 0, [[W, 128], [H * W, C], [1, W]]))
        refC = pr.tile([128, W * C], F16, tag="refC")  # [y,(x,c)]
        nc.vector.tensor_copy(
            out=_ap(refC[:], 0, [[W * C, 128], [C, W], [1, C]]),
            in_=_ap(tr[:], 0, [[C * W, 128], [1, W], [W, C]]))
        refPadC = pr.tile([128, PADX * C], F16, tag="refPadC")
        nc.vector.memset(refPadC[:], 0.0)
        nc.scalar.copy(out=refPadC[:, 2 * C:(2 + W) * C], in_=refC[:])
        refSC = {}
        for ty in (0, 1, 3, 4):
            t = pr.tile([128, PADX * C], F16, tag=f"refSC{ty}")
            k = ty - 2
            nc.vector.memset(t[:], 0.0)
            if k < 0:
                nc.sync.dma_start(out=t[-k:128, :], in_=refPadC[0:128 + k, :])
            else:
                nc.sync.dma_start(out=t[0:128 - k, :], in_=refPadC[k:128, :])
            refSC[ty] = t
        refSC[2] = refPadC
        sqt = pr.tile([128, W * C], F16, tag="sqt")
        nc.vector.tensor_tensor(out=sqt[:], in0=refC[:], in1=refC[:],
                                op=OP.mult)
        r2 = pr.tile([128, W], F16, tag="r2")
        nc.vector.tensor_reduce(
            out=r2[:], in_=_ap(sqt[:], 0, [[W * C, 128], [C, W], [1, C]]),
            axis=AX.X, op=OP.add)
        r2p = pr.tile([128, PADX], F16, tag="r2p")
        nc.vector.memset(r2p[:], 0.0)
        nc.scalar.copy(out=r2p[:, 2:2 + W], in_=r2[:])
        r2SC = {}
        for ty in (0, 1, 3, 4):
            t = pr.tile([128, PADX], F16, tag=f"r2SC{ty}")
            k = ty - 2
            nc.vector.memset(t[:], 0.0)
            if k < 0:
                nc.sync.dma_start(out=t[-k:128, :], in_=r2p[0:128 + k, :])
            else:
                nc.sync.dma_start(out=t[0:128 - k, :], in_=r2p[k:128, :])
            r2SC[ty] = t
        r2SC[2] = r2p

        wfvol = pp.tile([128, W * 25], F16, tag="wfvol")  # [y,(x,t25)]
        for ty in range(5):
            for t25 in range(5):
                t = ty * 5 + t25
                wf_ap = _ap(wfvol[:], t, [[W * 25, 128], [25, W], [1, 1]])
                if t == 12:
                    nc.vector.memset(wf_ap, 0.0)
                    continue
                xprod = pr.tile([128, W * C], F16, tag="xprod")
                nc.vector.tensor_tensor(
                    out=xprod[:],
                    in0=_ap(refSC[ty][:], t25 * C,
                            [[PADX * C, 128], [C, W], [1, C]]),
                    in1=refC[:], op=OP.mult)
                xd = pr.tile([128, W], F16, tag="xd")
                nc.vector.tensor_reduce(
                    out=xd[:],
                    in_=_ap(xprod[:], 0, [[W * C, 128], [C, W], [1, C]]),
                    axis=AX.X, op=OP.add)
                wf2 = pr.tile([128, W], F16, tag="wf2")
                nc.vector.scalar_tensor_tensor(
                    out=wf2[:], in0=xd[:], scalar=-2.0, in1=r2[:],
                    op0=OP.mult, op1=OP.add)
                wf2b = pr.tile([128, W], F16, tag="wf2b")
                nc.vector.tensor_tensor(
                    out=wf2b[:], in0=wf2[:],
                    in1=_ap(r2SC[ty][:], t25, [[PADX, 128], [1, W]]),
                    op=OP.add)
                wf2c = pr.tile([128, W], F16, tag="wf2c")
                nc.vector.tensor_scalar(
                    out=wf2c[:], in0=wf2b[:], scalar1=0.0, scalar2=None,
                    op0=OP.max)
                nc.scalar.activation(out=wf_ap, in_=wf2c[:], func=AF.Sqrt)
        prep.__exit__(None, None, None)

        # ---------------- work pools ----------------
        wp = pool("workpix", 1)    # small [128, W] pixel tiles
        wB = pool("workblk", 1)    # block-layout f16 tiles
        gp = pool("gath", 2)       # gather destinations
        cp = pool("cost", 2)       # cost conversion tiles

        depD = pp.tile([128, W * DPC], F32, tag="depD")
        aggT = pp.tile([128, W * DPC], F32, tag="aggT")
        costp0 = []
        for di in range(DPC):
            cpt = pp.tile([128, PADX], F16, tag=f"costp0_{di}")
            nc.vector.memset(cpt[:], 0.0)
            costp0.append(cpt)

        tab_loads = []  # DMA handles for current view table load
        diff_w = {}     # di -> dscr write DMA handle
        fscr_w = {}     # (slot) -> fscr write DMA handle

        def coords(vi, depf):
            """Pixel-layout coords for view vi: returns (idx0, idx1, fx, fy)
            with fx/fy as [y,x] f16 tiles."""
            def arow(r):
                return _ap(amapL[:], (vi * 3 + r) * W,
                           [[NV * 3 * W, 128], [1, W]])
            mx = wp.tile([128, W], F32, tag="mx")
            my = wp.tile([128, W], F32, tag="my")
            dn = wp.tile([128, W], F32, tag="dn")
            nc.vector.tensor_tensor(out=mx[:], in0=arow(0), in1=depf[:],
                                    op=OP.mult)
            nc.vector.tensor_tensor(out=my[:], in0=arow(1), in1=depf[:],
                                    op=OP.mult)
            nc.vector.tensor_tensor(out=dn[:], in0=arow(2), in1=depf[:],
                                    op=OP.mult)
            nx = wp.tile([128, W], F32, tag="nx")
            ny = wp.tile([128, W], F32, tag="ny")
            dnt = wp.tile([128, W], F32, tag="dnt")
            nc.vector.tensor_scalar(out=nx[:], in0=mx[:],
                                    scalar1=tv(vi * 3 + 0), scalar2=None,
                                    op0=OP.add)
            nc.vector.tensor_scalar(out=ny[:], in0=my[:],
                                    scalar1=tv(vi * 3 + 1), scalar2=None,
                                    op0=OP.add)
            nc.vector.tensor_scalar(out=dnt[:], in0=dn[:],
                                    scalar1=tv(vi * 3 + 2), scalar2=None,
                                    op0=OP.add)
            rec = wp.tile([128, W], F32, tag="rec")
            nc.vector.reciprocal(out=rec[:], in_=dnt[:])
            gxB = wp.tile([128, W], F32, tag="gxB")
            gyB = wp.tile([128, W], F32, tag="gyB")
            nc.vector.tensor_tensor(out=gxB[:], in0=nx[:], in1=rec[:],
                                    op=OP.mult)
            nc.vector.tensor_tensor(out=gyB[:], in0=ny[:], in1=rec[:],
                                    op=OP.mult)
            # += BIG - 0.5
            nc.scalar.activation(out=gxB[:], in_=gxB[:], func=AF.Identity,
                                 bias=tv(6))
            nc.scalar.activation(out=gyB[:], in_=gyB[:], func=AF.Identity,
                                 bias=tv(6))

            def floorfrac(g, nm):
                xi0 = wp.tile([128, W], mybir.dt.int32, tag=f"i{nm}")
                nc.vector.tensor_copy(out=xi0[:], in_=g[:])
                xf = wp.tile([128, W], F32, tag=f"xf{nm}")
                nc.scalar.copy(out=xf[:], in_=xi0[:])
                gt = wp.tile([128, W], F32, tag=f"gt{nm}")
                nc.vector.tensor_tensor(out=gt[:], in0=xf[:], in1=g[:],
                                        op=OP.is_gt)
                xif = wp.tile([128, W], F32, tag=f"xif{nm}")
                nc.vector.tensor_tensor(out=xif[:], in0=xf[:], in1=gt[:],
                                        op=OP.subtract)
                fr = wp.tile([128, W], F32, tag=f"fr{nm}")
                nc.vector.tensor_tensor(out=fr[:], in0=g[:], in1=xif[:],
                                        op=OP.subtract)
                return xif, fr

            xif, fx = floorfrac(gxB, "x")
            yif, fy = floorfrac(gyB, "y")
            xiC = wp.tile([128, W], F32, tag="xiC")
            yiC = wp.tile([128, W], F32, tag="yiC")
            nc.vector.tensor_scalar(out=xiC[:], in0=xif[:],
                                    scalar1=BIG + 129.0, scalar2=BIG - 2.0,
                                    op0=OP.min, op1=OP.max)
            nc.vector.tensor_scalar(out=yiC[:], in0=yif[:],
                                    scalar1=BIG + 129.0, scalar2=BIG - 2.0,
                                    op0=OP.min, op1=OP.max)
            posF = wp.tile([128, W], F32, tag="posF")
            nc.vector.scalar_tensor_tensor(
                out=posF[:], in0=yiC[:], scalar=float(PW), in1=xiC[:],
                op0=OP.mult, op1=OP.add)
            posK = wp.tile([128, W], F32, tag="posK")
            nc.vector.tensor_scalar(out=posK[:], in0=posF[:],
                                    scalar1=float(KPOS), scalar2=None,
                                    op0=OP.add)
            pos2 = wp.tile([128, W], F32, tag="pos2")
            nc.vector.tensor_scalar(out=pos2[:], in0=posK[:],
                                    scalar1=float(PW), scalar2=None,
                                    op0=OP.add)
            idx0 = wp.tile([128, W], I16, tag="idx0")
            nc.vector.tensor_copy(out=idx0[:], in_=posK[:])
            idx1 = wp.tile([128, W], I16, tag="idx1")
            nc.vector.tensor_copy(out=idx1[:], in_=pos2[:])
            fx16 = wp.tile([128, W], F16, tag="fx16")
            fy16 = wp.tile([128, W], F16, tag="fy16")
            nc.scalar.copy(out=fx16[:], in_=fx[:])
            nc.scalar.copy(out=fy16[:], in_=fy[:])
            return idx0, idx1, fx16, fy16, posF, fx, fy

        def frac_to_block(f16tile, slot):
            """fx/fy [y,x] f16 -> block layout [128, NPIX]: PE transpose to
            [x,y], spill block-ordered to DRAM, read back replicated."""
            pst = psp.tile([128, W], F16, tag="pst")
            nc.tensor.transpose(out=pst[:], in_=f16tile[:],
                                identity=identT[:])
            fT = wp.tile([128, W], F16, tag=f"fT{slot}")
            nc.scalar.copy(out=fT[:], in_=pst[:])
            # fscr[slot] holds block order: k*2048 + x*16 + y_l
            wdma = nc.sync.dma_start(
                out=_ap(fscr.ap(), slot * H * W,
                        [[16, 128], [2048, 8], [1, 16]]),
                in_=fT[:])
            fB = wB.tile([128, NPIX], F16, tag=f"fB{slot}")
            rdmas = []
            for k in range(8):
                rdma = nc.sync.dma_start(
                    out=fB[16 * k:16 * k + 16, :],
                    in_=_ap(fscr.ap(), slot * H * W + k * NPIX,
                            [[0, 16], [1, NPIX]]))
                add_dep_helper(rdma.ins, wdma.ins, reason="fscr write->read")
                rdmas.append(rdma)
            for prev in fscr_w.get(slot, ()):
                # new write must wait for the previous reads of this slot
                add_dep_helper(wdma.ins, prev.ins, reason="fscr read->write")
            fscr_w[slot] = rdmas
            return fB

        def warp_view(vi, di, depf):
            """Gather + bilinear for (vi, di): returns warped [128,NPIX] f16."""
            idx0, idx1, fx16, fy16, posF, fx, fy = coords(vi, depf)
            fxB = frac_to_block(fx16, 0)
            fyB = frac_to_block(fy16, 1)
            G0 = gp.tile([128, NPIX * 2], F16, tag="G0")
            G1 = gp.tile([128, NPIX * 2], F16, tag="G1")
            g0 = nc.gpsimd.ap_gather(
                out_ap=_ap(G0[:], 0, [[NPIX * 2, 128], [2, NPIX], [1, 2]]),
                in_ap=_ap(tabT[:], 0, [[NE * 2, 128], [2, NE], [1, 2]]),
                idxs_ap=idx0[:],
                channels=128, num_elems=NE, d=2, num_idxs=NPIX)
            g1 = nc.gpsimd.ap_gather(
                out_ap=_ap(G1[:], 0, [[NPIX * 2, 128], [2, NPIX], [1, 2]]),
                in_ap=_ap(tabT[:], 0, [[NE * 2, 128], [2, NE], [1, 2]]),
                idxs_ap=idx1[:],
                channels=128, num_elems=NE, d=2, num_idxs=NPIX)
            for dma_i in tab_loads:
                add_dep_helper(g0.ins, dma_i.ins, reason="table before gather")
                add_dep_helper(g1.ins, dma_i.ins, reason="table before gather")
            # t = G0 + fy*(G1-G0)  (pairs)
            pview = [[NPIX * 2, 128], [2, NPIX], [1, 2]]

            def dup(t):
                return _ap(t[:], 0, [[NPIX, 128], [1, NPIX], [0, 2]])

            t0 = wB.tile([128, NPIX * 2], F16, tag="t8a")
            t1 = wB.tile([128, NPIX * 2], F16, tag="t8b")
            nc.vector.tensor_tensor(out=t1[:], in0=G1[:], in1=G0[:],
                                    op=OP.subtract)
            nc.vector.tensor_tensor(out=_ap(t0[:], 0, pview),
                                    in0=_ap(t1[:], 0, pview),
                                    in1=dup(fyB), op=OP.mult)
            nc.vector.tensor_tensor(out=t0[:], in0=t0[:], in1=G0[:],
                                    op=OP.add)
            # warped = t_even + fx*(t_odd - t_even) ; scratch in t1
            sview = [[NPIX * 2, 128], [2, NPIX], [1, 1]]
            wtmp = _ap(t1[:], 0, [[NPIX * 2, 128], [1, NPIX]])
            nc.vector.tensor_tensor(out=wtmp,
                                    in0=_ap(t0[:], 1, sview),
                                    in1=_ap(t0[:], 0, sview),
                                    op=OP.subtract)
            warped = wB.tile([128, NPIX], F16, tag="warped")
            nc.vector.tensor_tensor(out=warped[:], in0=wtmp,
                                    in1=fxB[:], op=OP.mult)
            nc.vector.tensor_tensor(out=warped[:], in0=warped[:],
                                    in1=_ap(t0[:], 0, sview), op=OP.add)
            if debug and di == 0:
                nc.sync.dma_start(
                    out=_ap(dbg_warp.ap(), vi * 128 * NPIX,
                            [[NPIX, 128], [1, NPIX]]),
                    in_=warped[:])
                dc = wp.tile([128, W], F32, tag="dcoord")
                nc.vector.tensor_copy(out=dc[:], in_=posF[:])
                nc.sync.dma_start(out=dbg_coord.ap()[2 * vi], in_=dc[:])
                dc2 = wp.tile([128, W], F32, tag="dcoord2")
                nc.vector.tensor_copy(out=dc2[:], in_=fx[:])
                nc.sync.dma_start(out=dbg_coord.ap()[2 * vi + 1], in_=dc2[:])
            return warped

        cscr_state = {"n": 0, 0: [], 1: []}

        def cost_block_to_pixel(sq, dst_padded):
            """sq [128, NPIX] f16 -> PE c-reduce -> [y, x] window of
            dst_padded (writes cols 2..130) via a DRAM roundtrip."""
            cB = cp.tile([8, NPIX], F16, tag="cB")
            for grp in range(4):
                ps = psp.tile([8, 512], F32, tag="psc")
                for yl in range(4):
                    y_l = grp * 4 + yl
                    nc.tensor.matmul(
                        out=ps[:, yl * W:(yl + 1) * W],
                        lhsT=bselT[:, 0:8],
                        rhs=_ap(sq[:], y_l, [[NPIX, 128], [16, W]]),
                        start=True, stop=True)
                nc.scalar.copy(out=cB[:, grp * 512:(grp + 1) * 512],
                               in_=ps[:, :])
            # cB free order is now (y_l, x): block-pixel raster
            slot = cscr_state["n"] % 2
            cscr_state["n"] += 1
            wdma = nc.sync.dma_start(
                out=_ap(cscr.ap(), slot * H * W, [[NPIX, 8], [1, NPIX]]),
                in_=cB[:])
            for prev in cscr_state[slot]:
                add_dep_helper(wdma.ins, prev.ins, reason="cscr read->write")
            rdma = nc.sync.dma_start(
                out=_ap(dst_padded[:], 2, [[PADX, 128], [1, W]]),
                in_=_ap(cscr.ap(), slot * H * W, [[W, 128], [1, W]]))
            add_dep_helper(rdma.ins, wdma.ins, reason="cscr write->read")
            cscr_state[slot] = [rdma]

        # ================= view 0 phase =================
        tab_loads = []
        for k in range(8):
            d0 = nc.sync.dma_start(
                out=tabT[16 * k:16 * k + 16, :],
                in_=_ap(ptab.ap(), 0, [[NE * 2, 16], [1, NE * 2]]))
            tab_loads.append(d0)
        for di in range(DPC):
            depf = wp.tile([128, W], F32, tag="depf")
            nc.sync.dma_start(out=depf[:], in_=dep.ap()[di])
            warped = warp_view(0, di, depf)
            diffT = wB.tile([128, NPIX], F16, tag="diffT")
            nc.vector.tensor_tensor(out=diffT[:], in0=refB[:], in1=warped[:],
                                    op=OP.subtract)
            dw = nc.sync.dma_start(
                out=_ap(dscr.ap(), di * H * W * C, [[NPIX, 128], [1, NPIX]]),
                in_=diffT[:])
            diff_w[di] = dw
            sq = wB.tile([128, NPIX], F16, tag="sq")
            nc.vector.tensor_tensor(out=sq[:], in0=diffT[:], in1=diffT[:],
                                    op=OP.mult)
            cost_block_to_pixel(sq, costp0[di])

        # ================= view 1 phase =================
        # table overwrite waits for view-0 gathers via the tile-tracked
        # SBUF write-after-read hazard on tabT.
        tab_loads = []
        for k in range(8):
            d1 = nc.sync.dma_start(
                out=tabT[16 * k:16 * k + 16, :],
                in_=_ap(ptab.ap(), C * NE * 2,
                        [[NE * 2, 16], [1, NE * 2]]))
            tab_loads.append(d1)
        for di in range(DPC):
            depf = wp.tile([128, W], F32, tag="depf")
            nc.sync.dma_start(out=depf[:], in_=dep.ap()[di])
            nc.vector.tensor_copy(
                out=_ap(depD[:], di, [[W * DPC, 128], [DPC, W], [1, 1]]),
                in_=depf[:])
            warped = warp_view(1, di, depf)
            diffT = wB.tile([128, NPIX], F16, tag="diffT")
            rd = nc.sync.dma_start(
                out=diffT[:],
                in_=_ap(dscr.ap(), di * H * W * C, [[NPIX, 128], [1, NPIX]]))
            add_dep_helper(rd.ins, diff_w[di].ins, reason="dscr write->read")
            nc.vector.tensor_tensor(out=diffT[:], in0=diffT[:],
                                    in1=warped[:], op=OP.subtract)
            sq = wB.tile([128, NPIX], F16, tag="sq")
            nc.vector.tensor_tensor(out=sq[:], in0=diffT[:], in1=diffT[:],
                                    op=OP.mult)
            costp1 = cp.tile([128, PADX], F16, tag="costp1")
            cost_block_to_pixel(sq, costp1)
            # cost = sqrt(min(c0, c1)), padded [y, 132]
            costp = wp.tile([128, PADX], F16, tag="costp")
            nc.vector.memset(costp[:], 0.0)
            cmin = wp.tile([128, W], F16, tag="cmin")
            nc.vector.tensor_tensor(out=cmin[:],
                                    in0=costp0[di][:, 2:2 + W],
                                    in1=costp1[:, 2:2 + W], op=OP.min)
            nc.scalar.activation(out=costp[:, 2:2 + W], in_=cmin[:],
                                 func=AF.Sqrt)
            if debug:
                cdbg = wp.tile([128, W], F32, tag="cdbg")
                nc.scalar.copy(out=cdbg[:], in_=costp[:, 2:2 + W])
                nc.sync.dma_start(out=dbg_cost.ap()[di], in_=cdbg[:])
            costSC = {}
            for ty in (0, 1, 3, 4):
                t = wp.tile([128, PADX], F16, tag=f"costSC{ty}")
                k = ty - 2
                nc.vector.memset(t[:], 0.0)
                if k < 0:
                    nc.sync.dma_start(out=t[-k:128, :],
                                      in_=costp[0:128 + k, :])
                else:
                    nc.sync.dma_start(out=t[0:128 - k, :],
                                      in_=costp[k:128, :])
                costSC[ty] = t
            costSC[2] = costp
            # depth pads + shifts
            depp = wp.tile([128, PADX], F16, tag="depp")
            nc.vector.memset(depp[:], 0.0)
            nc.scalar.copy(out=depp[:, 2:2 + W], in_=depf[:])
            depSC = {}
            for ty in (0, 1, 3, 4):
                t = wp.tile([128, PADX], F16, tag=f"depSC{ty}")
                k = ty - 2
                nc.vector.memset(t[:], 0.0)
                if k < 0:
                    nc.sync.dma_start(out=t[-k:128, :],
                                      in_=depp[0:128 + k, :])
                else:
                    nc.sync.dma_start(out=t[0:128 - k, :],
                                      in_=depp[k:128, :])
                depSC[ty] = t
            depSC[2] = depp
            # aggregation: num/den over 25 taps
            num = wp.tile([128, W], F32, tag="num")
            den = wp.tile([128, W], F32, tag="den")
            for ty in range(5):
                dvol = wp.tile([128, W * 5], F16, tag="dvol")
                vvv = [[W * 5, 128], [5, W], [1, 5]]
                nc.vector.tensor_tensor(
                    out=_ap(dvol[:], 0, vvv),
                    in0=_ap(depSC[ty][:], 0, [[PADX, 128], [1, W], [1, 5]]),
                    in1=_ap(depp[:], 2, [[PADX, 128], [1, W], [0, 5]]),
                    op=OP.subtract)
                nc.scalar.activation(out=dvol[:], in_=dvol[:], func=AF.Abs)
                evol = wp.tile([128, W * 5], F16, tag="evol")
                nc.scalar.activation(out=evol[:], in_=dvol[:], func=AF.Exp,
                                     bias=tv(7))
                uvol = wp.tile([128, W * 5], F16, tag="uvol")
                nc.vector.tensor_tensor(
                    out=_ap(uvol[:], 0, vvv), in0=_ap(evol[:], 0, vvv),
                    in1=_ap(wfvol[:], ty * 5,
                            [[W * 25, 128], [25, W], [1, 5]]),
                    op=OP.mult)
                nvol = wp.tile([128, W * 5], F16, tag="nvol")
                nc.vector.tensor_tensor(
                    out=_ap(nvol[:], 0, vvv), in0=_ap(uvol[:], 0, vvv),
                    in1=_ap(costSC[ty][:], 0, [[PADX, 128], [1, W], [1, 5]]),
                    op=OP.mult)
                sty = wp.tile([128, W], F16, tag="sty")
                nc.vector.tensor_reduce(out=sty[:], in_=_ap(nvol[:], 0, vvv),
                                        axis=AX.X, op=OP.add)
                ety = wp.tile([128, W], F16, tag="ety")
                nc.vector.tensor_reduce(out=ety[:], in_=_ap(evol[:], 0, vvv),
                                        axis=AX.X, op=OP.add)
                if ty == 0:
                    nc.vector.tensor_copy(out=num[:], in_=sty[:])
                    nc.vector.tensor_copy(out=den[:], in_=ety[:])
                else:
                    nc.vector.tensor_tensor(out=num[:], in0=num[:],
                                            in1=sty[:], op=OP.add)
                    nc.vector.tensor_tensor(out=den[:], in0=den[:],
                                            in1=ety[:], op=OP.add)
            rden = wp.tile([128, W], F32, tag="rden")
            nc.vector.reciprocal(out=rden[:], in_=den[:])
            agg_ap = _ap(aggT[:], di, [[W * DPC, 128], [DPC, W], [1, 1]])
            nc.vector.tensor_tensor(out=agg_ap, in0=num[:], in1=rden[:],
                                    op=OP.mult)
            if debug:
                adbg = wp.tile([128, W], F32, tag="adbg")
                nc.vector.tensor_tensor(out=adbg[:], in0=num[:], in1=rden[:],
                                        op=OP.mult)
                nc.sync.dma_start(out=dbg_agg.ap()[di], in_=adbg[:])

        # ---------------- per-core softmax partials ----------------
        def aggap(di):
            return _ap(aggT[:], di, [[W * DPC, 128], [DPC, W], [1, 1]])

        def depap(di):
            return _ap(depD[:], di, [[W * DPC, 128], [DPC, W], [1, 1]])

        m = pp.tile([128, W], F32, tag="m")
        nc.vector.tensor_tensor(out=m[:], in0=aggap(0), in1=aggap(1),
                                op=OP.max)
        for di in range(2, DPC):
            nc.vector.tensor_tensor(out=m[:], in0=m[:], in1=aggap(di),
                                    op=OP.max)
        s0 = pp.tile([128, W], F32, tag="s0")
        s1 = pp.tile([128, W], F32, tag="s1")
        for di in range(DPC):
            t = wp.tile([128, W], F32, tag="et")
            nc.vector.tensor_tensor(out=t[:], in0=aggap(di), in1=m[:],
                                    op=OP.subtract)
            e = wp.tile([128, W], F32, tag="ee")
            nc.scalar.activation(out=e[:], in_=t[:], func=AF.Exp)
            t1 = wp.tile([128, W], F32, tag="t1")
            nc.vector.tensor_tensor(out=t1[:], in0=e[:], in1=depap(di),
                                    op=OP.mult)
            if di == 0:
                nc.vector.tensor_copy(out=s0[:], in_=e[:])
                nc.vector.tensor_copy(out=s1[:], in_=t1[:])
            else:
                nc.vector.tensor_tensor(out=s0[:], in0=s0[:], in1=e[:],
                                        op=OP.add)
                nc.vector.tensor_tensor(out=s1[:], in0=s1[:], in1=t1[:],
                                        op=OP.add)
        nc.sync.dma_start(out=out3.ap()[0], in_=m[:])
        nc.sync.dma_start(out=out3.ap()[1], in_=s0[:])
        nc.sync.dma_start(out=out3.ap()[2], in_=s1[:])

        for p in reversed(ctx_pools):
            p.__exit__(None, None, None)

    nc.compile()
    return nc


def host_prep(features, intrinsics, cam_to_world, depth_hypo):
    """Build the 8 per-core input maps."""
    features = np.asarray(features, dtype=np.float32)
    intrinsics = np.asarray(intrinsics, dtype=np.float32)
    cam_to_world = np.asarray(cam_to_world, dtype=np.float32)
    depth_hypo = np.asarray(depth_hypo, dtype=np.float32)

    ys, xs = np.meshgrid(np.arange(H, dtype=np.float32),
                         np.arange(W, dtype=np.float32), indexing="ij")
    bsel = np.zeros((128, 8), np.float16)
    for p in range(128):
        bsel[p, p // 16] = 1.0

    # padded pair tables per (v, b): [C, NE, 2] f16
    ptabs = {}
    for b in range(B):
        for vi in range(1, V):
            img = features[vi, b].astype(np.float16)  # [C, H, W]
            pad = np.zeros((C, PH, PW), np.float16)
            pad[:, 2:2 + H, 2:2 + W] = img
            pair = np.zeros((C, PH, PW, 2), np.float16)
            pair[..., 0] = pad
            flatp = pad.reshape(C, -1)
            pair.reshape(C, -1, 2)[:, :-1, 1] = flatp[:, 1:]
            ptabs[(vi - 1, b)] = pair.reshape(C, NE * 2)

    in_maps = []
    for k in range(NCORES):
        b = k // (NCORES // B)
        dlo = DPC * (k % (NCORES // B))
        amap = np.zeros((NV, 3, H, W), np.float32)
        tvv = np.zeros((8,), np.float32)
        for vi in range(1, V):
            src_w2c = np.linalg.inv(cam_to_world[vi, b])
            ref_w2c = np.linalg.inv(cam_to_world[0, b])
            src_KK = src_w2c.copy()
            src_KK[:3, :3] = intrinsics[vi, b]
            ref_KK = ref_w2c.copy()
            ref_KK[:3, :3] = intrinsics[0, b]
            proj = (src_KK @ src_w2c) @ np.linalg.inv(ref_KK @ ref_w2c)
            rot, trans = proj[:3, :3], proj[:3, 3]
            A = (rot[:, 0:1, None] * xs[None] + rot[:, 1:2, None] * ys[None]
                 + rot[:, 2:3, None])  # [3, H, W]
            v = vi - 1
            amap[v, 0] = A[0] * SCALE
            amap[v, 1] = A[1] * SCALE
            amap[v, 2] = A[2]
            tvv[v * 3 + 0] = trans[0] * SCALE
            tvv[v * 3 + 1] = trans[1] * SCALE
            tvv[v * 3 + 2] = trans[2]
        tvv[6] = BIG - 0.5
        tvv[7] = ESHIFT
        ptab = np.stack([ptabs[(0, b)], ptabs[(1, b)]])  # [NV, C, NE*2]
        ref16 = features[0, b].astype(np.float16)        # [C, H, W]
        # refB[16k+c, x*16+y_l] = ref16[c, 16k+y_l, x]
        refB = np.ascontiguousarray(
            ref16.reshape(C, 8, 16, W).transpose(1, 0, 3, 2).reshape(
                128, NPIX))
        in_maps.append({
            "ptab": np.ascontiguousarray(ptab.reshape(NV, C * NE * 2)),
            "refF": np.ascontiguousarray(ref16),
            "refBin": refB,
            "ident": np.eye(128, dtype=np.float16),
            "dep": np.ascontiguousarray(depth_hypo[b, dlo:dlo + DPC]),
            "amap": np.ascontiguousarray(amap),
            "tvec": np.tile(tvv[None, :], (128, 1)).astype(np.float32),
            "bsel": bsel,
        })
    return in_maps


def host_combine(results):
    """Merge per-core softmax partials (m, s0, s1) into [B, H, W]."""
    out = np.zeros((B, H, W), np.float32)
    per_b = NCORES // B
    for b in range(B):
        parts = [np.asarray(results[b * per_b + j]["out3"])
                 for j in range(per_b)]
        ms = np.stack([p[0] for p in parts])
        M = ms.max(axis=0)
        S0 = np.zeros((H, W), np.float64)
        S1 = np.zeros((H, W), np.float64)
        for p in parts:
            w = np.exp(p[0] - M)
            S0 += w * p[1]
            S1 += w * p[2]
        out[b] = (S1 / S0).astype(np.float32)
    return out


def _run_sim(nc, in_maps, outs=("out3",)):
    from concourse.bass_interp import CoreSim
    results = []
    for core in range(NCORES):
        sim = CoreSim(nc, trace=False)
        for k, v in in_maps[core].items():
            sim.tensor(k)[:] = v
        sim.simulate()
        results.append({o: np.array(sim.tensor(o)) for o in outs})
    return results


def kernel(**inputs):
    if "nc" not in _cached:
        _cached["nc"] = build_program()
    nc = _cached["nc"]
    in_maps = host_prep(**inputs)
    if _cached.get("hw_broken"):
        return host_combine(_run_sim(nc, in_maps))
    try:
        res = run_bass_kernel_spmd(nc, in_maps, core_ids=list(range(NCORES)))
        return host_combine(res.results)
    except Exception:
        _cached["hw_broken"] = True
        return host_combine(_run_sim(nc, in_maps))


if __name__ == "__main__":
    import reference
    inp = reference.setup_inputs()
    inp = {k: np.asarray(v) for k, v in inp.items()}
    out = kernel(**inp)
    print("kernel out", out.shape, out.dtype)


# revision 3
# speedup vs baseline: 22.7288x; 1.3590x over previous
"""Trainium2 Bass kernel for nn_DepthPrediction — ap_gather version.

Strategy (8 NeuronCores, SPMD):
  - Shard: core k handles batch b = k//4 and depth planes [8*(k%4), +8).
  - Warp via gpsimd ap_gather (SBUF->SBUF ucode gather; the SWDGE
    dma_gather path aborts on this runtime).  Layout: 128 partitions =
    8 blocks x 16 channels; block k owns image rows [16k, 16k+16); free
    axis = 2048 pixels in (x, y_local) order, which makes the ap_gather
    index wrap coincide with the plain [y, x] pixel layout.
  - Table: host-built padded pair table per view: tab[pos] =
    (Fpad[pos], Fpad[pos+1]), pos = r*132 + c, r in [0,133), c in
    [0,132); r = y0+2, c = x0+2, zero border.  One gather (d=2) per
    bilinear row pair: idx -> (v00, v01), idx+132 -> (v10, v11).
  - One view table resident at a time; per-depth diff (ref - cum)
    spills to DRAM between the two view phases.
  - Cost: PE matmul with a block-selector matrix reduces over the 16
    channel partitions; SBUF->SBUF DMA reshapes [8, 2048] -> [y, x].
  - Aggregation (5x5 adaptive) + per-core softmax partials in pixel
    layout, as in the dma_gather version.
  - Host: 4x4 matrix algebra, table/shard packing, log-sum-exp merge
    of the 4-way softmax partials per batch.

Self-contained: hardcodes all shapes from the problem spec.
"""

import numpy as np

import concourse.bacc as bacc
import concourse.bass as bass
import concourse.mybir as mybir
import concourse.tile as tile
from concourse.bass_utils import run_bass_kernel_spmd
from concourse.tile_rust import add_dep_helper

F32 = mybir.dt.float32
F16 = mybir.dt.float16
I16 = mybir.dt.int16

AF = mybir.ActivationFunctionType
OP = mybir.AluOpType
AX = mybir.AxisListType

# problem shapes
V, B, C, H, W, D = 3, 2, 16, 128, 128, 32
NCORES = 8
DPC = D // (NCORES // B)  # depth planes per core = 8
NV = V - 1                # src views = 2

PW = 132                  # padded cols: x0 in [-2, 129] -> c = x0+2
PH = 133                  # padded rows: y0 in [-2, 129] -> r = y0+2; +1 row
NE = PH * PW              # 17556 table positions
NPIX = 16 * W             # pixels per partition block = 2048
PADX = W + 4              # x-padded pixel tiles for 5-tap aggregation
BIG = 1024.0              # float floor-shift
KPOS = -(BIG - 2.0) * PW - (BIG - 2.0)
SCALE = W / (W - 1.0)     # grid_sample align_corners=False fold
ESHIFT = -9.0             # exp(|dnb-d|) stabilization shift

_cached = {}


def _ap(base, off, dims):
    """Raw AP on the same tensor as `base` (an AP), offset in elements."""
    return bass.AP(base.tensor, base.offset + off, dims)


def build_program(debug=False):
    nc = bacc.Bacc("TRN2", target_bir_lowering=False, debug=False,
                   num_devices=NCORES)

    ptab = nc.dram_tensor("ptab", [NV, C * NE * 2], F16,
                          kind="ExternalInput")
    refF = nc.dram_tensor("refF", [C, H, W], F16, kind="ExternalInput")
    dep = nc.dram_tensor("dep", [DPC, H, W], F32, kind="ExternalInput")
    amap = nc.dram_tensor("amap", [NV, 3, H, W], F32, kind="ExternalInput")
    tvec = nc.dram_tensor("tvec", [128, 8], F32, kind="ExternalInput")
    bsel = nc.dram_tensor("bsel", [128, 8], F16, kind="ExternalInput")
    refBin = nc.dram_tensor("refBin", [128, NPIX], F16, kind="ExternalInput")
    ident = nc.dram_tensor("ident", [128, 128], F16, kind="ExternalInput")
    out3 = nc.dram_tensor("out3", [3, H, W], F32, kind="ExternalOutput")
    # DRAM scratch
    fscr = nc.dram_tensor("fscr", [2, H * W], F16, kind="Internal")
    cscr = nc.dram_tensor("cscr", [2, H * W], F16, kind="Internal")
    dscr = nc.dram_tensor("dscr", [DPC, H * W * C], F16, kind="Internal")
    if debug:
        dbg_warp = nc.dram_tensor("dbg_warp", [NV, 128, NPIX], F16,
                                  kind="ExternalOutput")
        dbg_cost = nc.dram_tensor("dbg_cost", [DPC, H, W], F32,
                                  kind="ExternalOutput")
        dbg_agg = nc.dram_tensor("dbg_agg", [DPC, H, W], F32,
                                 kind="ExternalOutput")
        dbg_coord = nc.dram_tensor("dbg_coord", [4, H, W], F32,
                                   kind="ExternalOutput")

    with nc.allow_low_precision("fp16 pipeline by design"), \
            tile.TileContext(nc) as tc:
        ctx_pools = []

        def pool(name, bufs=1, **kw):
            p = tc.tile_pool(name=name, bufs=bufs, **kw)
            ctx_pools.append(p)
            return p.__enter__()

        pp = pool("persist", 1)
        psp = pool("psum", 2, space="PSUM")

        # ---------------- persistent loads ----------------
        amapL = pp.tile([128, NV * 3 * W], F32, tag="amapL")  # [y,(v,row,x)]
        nc.sync.dma_start(out=amapL[:], in_=_ap(
            amap.ap(), 0, [[W, 128], [3 * H * W, NV], [H * W, 3], [1, W]]))
        tvecT = pp.tile([128, 8], F32, tag="tvecT")
        nc.sync.dma_start(out=tvecT[:], in_=tvec.ap())
        bselT = pp.tile([128, 8], F16, tag="bselT")
        nc.sync.dma_start(out=bselT[:], in_=bsel.ap())
        # ref features in block layout [p=(k,c), i=(x,y_l)] (host-permuted)
        refB = pp.tile([128, NPIX], F16, tag="refB")
        nc.sync.dma_start(out=refB[:], in_=refBin.ap())
        identT = pp.tile([128, 128], F16, tag="identT")
        nc.sync.dma_start(out=identT[:], in_=ident.ap())
        # view table (one view at a time)
        tabT = pp.tile([128, NE * 2], F16, tag="tabT")

        def tv(col):  # [128,1] per-partition scalar AP
            return tvecT[:, col:col + 1]

        # ---------------- pixel-layout ref prep (w_feat volume) ----------
        prep = tc.tile_pool(name="prep", bufs=1)
        pr = prep.__enter__()
        tr = pr.tile([128, C * W], F16, tag="tr")  # [y,(c,x)]
        nc.sync.dma_start(out=tr[:], in_=_ap(
            refF.ap(), 0, [[W, 128], [H * W, C], [1, W]]))
        refC = pr.tile([128, W * C], F16, tag="refC")  # [y,(x,c)]
        nc.vector.tensor_copy(
            out=_ap(refC[:], 0, [[W * C, 128], [C, W], [1, C]]),
            in_=_ap(tr[:], 0, [[C * W, 128], [1, W], [W, C]]))
        refPadC = pr.tile([128, PADX * C], F16, tag="refPadC")
        nc.vector.memset(refPadC[:], 0.0)
        nc.scalar.copy(out=refPadC[:, 2 * C:(2 + W) * C], in_=refC[:])
        refSC = {}
        for ty in (0, 1, 3, 4):
            t = pr.tile([128, PADX * C], F16, tag=f"refSC{ty}")
            k = ty - 2
            nc.vector.memset(t[:], 0.0)
            if k < 0:
                nc.sync.dma_start(out=t[-k:128, :], in_=refPadC[0:128 + k, :])
            else:
                nc.sync.dma_start(out=t[0:128 - k, :], in_=refPadC[k:128, :])
            refSC[ty] = t
        refSC[2] = refPadC
        sqt = pr.tile([128, W * C], F16, tag="sqt")
        nc.vector.tensor_tensor(out=sqt[:], in0=refC[:], in1=refC[:],
                                op=OP.mult)
        r2 = pr.tile([128, W], F16, tag="r2")
        nc.vector.tensor_reduce(
            out=r2[:], in_=_ap(sqt[:], 0, [[W * C, 128], [C, W], [1, C]]),
            axis=AX.X, op=OP.add)
        r2p = pr.tile([128, PADX], F16, tag="r2p")
        nc.vector.memset(r2p[:], 0.0)
        nc.scalar.copy(out=r2p[:, 2:2 + W], in_=r2[:])
        r2SC = {}
        for ty in (0, 1, 3, 4):
            t = pr.tile([128, PADX], F16, tag=f"r2SC{ty}")
            k = ty - 2
            nc.vector.memset(t[:], 0.0)
            if k < 0:
                nc.sync.dma_start(out=t[-k:128, :], in_=r2p[0:128 + k, :])
            else:
                nc.sync.dma_start(out=t[0:128 - k, :], in_=r2p[k:128, :])
            r2SC[ty] = t
        r2SC[2] = r2p

        wfvol = pp.tile([128, W * 25], F16, tag="wfvol")  # [y,(x,t25)]
        for ty in range(5):
            for t25 in range(5):
                t = ty * 5 + t25
                wf_ap = _ap(wfvol[:], t, [[W * 25, 128], [25, W], [1, 1]])
                if t == 12:
                    nc.vector.memset(wf_ap, 0.0)
                    continue
                xprod = pr.tile([128, W * C], F16, tag="xprod")
                nc.vector.tensor_tensor(
                    out=xprod[:],
                    in0=_ap(refSC[ty][:], t25 * C,
                            [[PADX * C, 128], [C, W], [1, C]]),
                    in1=refC[:], op=OP.mult)
                xd = pr.tile([128, W], F16, tag="xd")
                nc.vector.tensor_reduce(
                    out=xd[:],
                    in_=_ap(xprod[:], 0, [[W * C, 128], [C, W], [1, C]]),
                    axis=AX.X, op=OP.add)
                wf2 = pr.tile([128, W], F16, tag="wf2")
                nc.vector.scalar_tensor_tensor(
                    out=wf2[:], in0=xd[:], scalar=-2.0, in1=r2[:],
                    op0=OP.mult, op1=OP.add)
                wf2b = pr.tile([128, W], F16, tag="wf2b")
                nc.vector.tensor_tensor(
                    out=wf2b[:], in0=wf2[:],
                    in1=_ap(r2SC[ty][:], t25, [[PADX, 128], [1, W]]),
                    op=OP.add)
                wf2c = pr.tile([128, W], F16, tag="wf2c")
                nc.vector.tensor_scalar(
                    out=wf2c[:], in0=wf2b[:], scalar1=0.0, scalar2=None,
                    op0=OP.max)
                nc.scalar.activation(out=wf_ap, in_=wf2c[:], func=AF.Sqrt)
        prep.__exit__(None, None, None)

        # ---------------- work pools ----------------
        wp = pool("workpix", 1)    # small [128, W] pixel tiles
        wB = pool("workblk", 1)    # block-layout f16 tiles
        gp = pool("gath", 2)       # gather destinations
        cp = pool("cost", 2)       # cost conversion tiles

        depD = pp.tile([128, W * DPC], F32, tag="depD")
        aggT = pp.tile([128, W * DPC], F32, tag="aggT")
        costp0 = []
        for di in range(DPC):
            cpt = pp.tile([128, PADX], F16, tag=f"costp0_{di}")
            nc.vector.memset(cpt[:], 0.0)
            costp0.append(cpt)

        tab_loads = []  # DMA handles for current view table load
        diff_w = {}     # di -> dscr write DMA handle
        fscr_w = {}     # (slot) -> fscr write DMA handle

        def coords(vi, depf):
            """Pixel-layout coords for view vi: returns (idx0, idx1, fx, fy)
            with fx/fy as [y,x] f16 tiles."""
            def arow(r):
                return _ap(amapL[:], (vi * 3 + r) * W,
                           [[NV * 3 * W, 128], [1, W]])
            mx = wp.tile([128, W], F32, tag="mx")
            my = wp.tile([128, W], F32, tag="my")
            dn = wp.tile([128, W], F32, tag="dn")
            nc.vector.tensor_tensor(out=mx[:], in0=arow(0), in1=depf[:],
                                    op=OP.mult)
            nc.vector.tensor_tensor(out=my[:], in0=arow(1), in1=depf[:],
                                    op=OP.mult)
            nc.vector.tensor_tensor(out=dn[:], in0=arow(2), in1=depf[:],
                                    op=OP.mult)
            nx = wp.tile([128, W], F32, tag="nx")
            ny = wp.tile([128, W], F32, tag="ny")
            dnt = wp.tile([128, W], F32, tag="dnt")
            nc.vector.tensor_scalar(out=nx[:], in0=mx[:],
                                    scalar1=tv(vi * 3 + 0), scalar2=None,
                                    op0=OP.add)
            nc.vector.tensor_scalar(out=ny[:], in0=my[:],
                                    scalar1=tv(vi * 3 + 1), scalar2=None,
                                    op0=OP.add)
            nc.vector.tensor_scalar(out=dnt[:], in0=dn[:],
                                    scalar1=tv(vi * 3 + 2), scalar2=None,
                                    op0=OP.add)
            rec = wp.tile([128, W], F32, tag="rec")
            nc.vector.reciprocal(out=rec[:], in_=dnt[:])
            gxB = wp.tile([128, W], F32, tag="gxB")
            gyB = wp.tile([128, W], F32, tag="gyB")
            nc.vector.tensor_tensor(out=gxB[:], in0=nx[:], in1=rec[:],
                                    op=OP.mult)
            nc.vector.tensor_tensor(out=gyB[:], in0=ny[:], in1=rec[:],
                                    op=OP.mult)
            # += BIG - 0.5
            nc.scalar.activation(out=gxB[:], in_=gxB[:], func=AF.Identity,
                                 bias=tv(6))
            nc.scalar.activation(out=gyB[:], in_=gyB[:], func=AF.Identity,
                                 bias=tv(6))

            def floorfrac(g, nm):
                xi0 = wp.tile([128, W], mybir.dt.int32, tag=f"i{nm}")
                nc.vector.tensor_copy(out=xi0[:], in_=g[:])
                xf = wp.tile([128, W], F32, tag=f"xf{nm}")
                nc.scalar.copy(out=xf[:], in_=xi0[:])
                gt = wp.tile([128, W], F32, tag=f"gt{nm}")
                nc.vector.tensor_tensor(out=gt[:], in0=xf[:], in1=g[:],
                                        op=OP.is_gt)
                xif = wp.tile([128, W], F32, tag=f"xif{nm}")
                nc.vector.tensor_tensor(out=xif[:], in0=xf[:], in1=gt[:],
                                        op=OP.subtract)
                fr = wp.tile([128, W], F32, tag=f"fr{nm}")
                nc.vector.tensor_tensor(out=fr[:], in0=g[:], in1=xif[:],
                                        op=OP.subtract)
                return xif, fr

            xif, fx = floorfrac(gxB, "x")
            yif, fy = floorfrac(gyB, "y")
            xiC = wp.tile([128, W], F32, tag="xiC")
            yiC = wp.tile([128, W], F32, tag="yiC")
            nc.vector.tensor_scalar(out=xiC[:], in0=xif[:],
                                    scalar1=BIG + 129.0, scalar2=BIG - 2.0,
                                    op0=OP.min, op1=OP.max)
            nc.vector.tensor_scalar(out=yiC[:], in0=yif[:],
                                    scalar1=BIG + 129.0, scalar2=BIG - 2.0,
                                    op0=OP.min, op1=OP.max)
            posF = wp.tile([128, W], F32, tag="posF")
            nc.vector.scalar_tensor_tensor(
                out=posF[:], in0=yiC[:], scalar=float(PW), in1=xiC[:],
                op0=OP.mult, op1=OP.add)
            posK = wp.tile([128, W], F32, tag="posK")
            nc.vector.tensor_scalar(out=posK[:], in0=posF[:],
                                    scalar1=float(KPOS), scalar2=None,
                                    op0=OP.add)
            pos2 = wp.tile([128, W], F32, tag="pos2")
            nc.vector.tensor_scalar(out=pos2[:], in0=posK[:],
                                    scalar1=float(PW), scalar2=None,
                                    op0=OP.add)
            idx0 = wp.tile([128, W], I16, tag="idx0")
            nc.vector.tensor_copy(out=idx0[:], in_=posK[:])
            idx1 = wp.tile([128, W], I16, tag="idx1")
            nc.vector.tensor_copy(out=idx1[:], in_=pos2[:])
            fx16 = wp.tile([128, W], F16, tag="fx16")
            fy16 = wp.tile([128, W], F16, tag="fy16")
            nc.scalar.copy(out=fx16[:], in_=fx[:])
            nc.scalar.copy(out=fy16[:], in_=fy[:])
            return idx0, idx1, fx16, fy16, posF, fx, fy

        def frac_to_block(f16tile, slot):
            """fx/fy [y,x] f16 -> block layout [128, NPIX]: PE transpose to
            [x,y], spill block-ordered to DRAM, read back replicated."""
            pst = psp.tile([128, W], F16, tag="pst")
            nc.tensor.transpose(out=pst[:], in_=f16tile[:],
                                identity=identT[:])
            fT = wp.tile([128, W], F16, tag=f"fT{slot}")
            nc.scalar.copy(out=fT[:], in_=pst[:])
            # fscr[slot] holds block order: k*2048 + x*16 + y_l
            wdma = nc.sync.dma_start(
                out=_ap(fscr.ap(), slot * H * W,
                        [[16, 128], [2048, 8], [1, 16]]),
                in_=fT[:])
            fB = wB.tile([128, NPIX], F16, tag=f"fB{slot}")
            rdmas = []
            for k in range(8):
                rdma = nc.sync.dma_start(
                    out=fB[16 * k:16 * k + 16, :],
                    in_=_ap(fscr.ap(), slot * H * W + k * NPIX,
                            [[0, 16], [1, NPIX]]))
                add_dep_helper(rdma.ins, wdma.ins, reason="fscr write->read")
                rdmas.append(rdma)
            for prev in fscr_w.get(slot, ()):
                # new write must wait for the previous reads of this slot
                add_dep_helper(wdma.ins, prev.ins, reason="fscr read->write")
            fscr_w[slot] = rdmas
            return fB

        def warp_view(vi, di, depf):
            """Gather + bilinear for (vi, di): returns warped [128,NPIX] f16."""
            idx0, idx1, fx16, fy16, posF, fx, fy = coords(vi, depf)
            fxB = frac_to_block(fx16, 0)
            fyB = frac_to_block(fy16, 1)
            G0 = gp.tile([128, NPIX * 2], F16, tag="G0")
            G1 = gp.tile([128, NPIX * 2], F16, tag="G1")
            g0 = nc.gpsimd.ap_gather(
                out_ap=_ap(G0[:], 0, [[NPIX * 2, 128], [2, NPIX], [1, 2]]),
                in_ap=_ap(tabT[:], 0, [[NE * 2, 128], [2, NE], [1, 2]]),
                idxs_ap=idx0[:],
                channels=128, num_elems=NE, d=2, num_idxs=NPIX)
            g1 = nc.gpsimd.ap_gather(
                out_ap=_ap(G1[:], 0, [[NPIX * 2, 128], [2, NPIX], [1, 2]]),
                in_ap=_ap(tabT[:], 0, [[NE * 2, 128], [2, NE], [1, 2]]),
                idxs_ap=idx1[:],
                channels=128, num_elems=NE, d=2, num_idxs=NPIX)
            for dma_i in tab_loads:
                add_dep_helper(g0.ins, dma_i.ins, reason="table before gather")
                add_dep_helper(g1.ins, dma_i.ins, reason="table before gather")
            # t = G0 + fy*(G1-G0)  (pairs)
            pview = [[NPIX * 2, 128], [2, NPIX], [1, 2]]

            def dup(t):
                return _ap(t[:], 0, [[NPIX, 128], [1, NPIX], [0, 2]])

            t0 = wB.tile([128, NPIX * 2], F16, tag="t8a")
            t1 = wB.tile([128, NPIX * 2], F16, tag="t8b")
            nc.vector.tensor_tensor(out=t1[:], in0=G1[:], in1=G0[:],
                                    op=OP.subtract)
            nc.vector.tensor_tensor(out=_ap(t0[:], 0, pview),
                                    in0=_ap(t1[:], 0, pview),
                                    in1=dup(fyB), op=OP.mult)
            nc.vector.tensor_tensor(out=t0[:], in0=t0[:], in1=G0[:],
                                    op=OP.add)
            # warped = t_even + fx*(t_odd - t_even) ; scratch in t1
            sview = [[NPIX * 2, 128], [2, NPIX], [1, 1]]
            wtmp = _ap(t1[:], 0, [[NPIX * 2, 128], [1, NPIX]])
            nc.vector.tensor_tensor(out=wtmp,
                                    in0=_ap(t0[:], 1, sview),
                                    in1=_ap(t0[:], 0, sview),
                                    op=OP.subtract)
            warped = wB.tile([128, NPIX], F16, tag="warped")
            nc.vector.tensor_tensor(out=warped[:], in0=wtmp,
                                    in1=fxB[:], op=OP.mult)
            nc.vector.tensor_tensor(out=warped[:], in0=warped[:],
                                    in1=_ap(t0[:], 0, sview), op=OP.add)
            if debug and di == 0:
                nc.sync.dma_start(
                    out=_ap(dbg_warp.ap(), vi * 128 * NPIX,
                            [[NPIX, 128], [1, NPIX]]),
                    in_=warped[:])
                dc = wp.tile([128, W], F32, tag="dcoord")
                nc.vector.tensor_copy(out=dc[:], in_=posF[:])
                nc.sync.dma_start(out=dbg_coord.ap()[2 * vi], in_=dc[:])
                dc2 = wp.tile([128, W], F32, tag="dcoord2")
                nc.vector.tensor_copy(out=dc2[:], in_=fx[:])
                nc.sync.dma_start(out=dbg_coord.ap()[2 * vi + 1], in_=dc2[:])
            return warped

        cscr_state = {"n": 0, 0: [], 1: []}

        def cost_block_to_pixel(sq, dst_padded):
            """sq [128, NPIX] f16 -> PE c-reduce -> [y, x] window of
            dst_padded (writes cols 2..130) via a DRAM roundtrip."""
            cB = cp.tile([8, NPIX], F16, tag="cB")
            for grp in range(4):
                ps = psp.tile([8, 512], F32, tag="psc")
                for yl in range(4):
                    y_l = grp * 4 + yl
                    nc.tensor.matmul(
                        out=ps[:, yl * W:(yl + 1) * W],
                        lhsT=bselT[:, 0:8],
                        rhs=_ap(sq[:], y_l, [[NPIX, 128], [16, W]]),
                        start=True, stop=True)
                nc.scalar.copy(out=cB[:, grp * 512:(grp + 1) * 512],
                               in_=ps[:, :])
            # cB free order is now (y_l, x): block-pixel raster
            slot = cscr_state["n"] % 2
            cscr_state["n"] += 1
            wdma = nc.sync.dma_start(
                out=_ap(cscr.ap(), slot * H * W, [[NPIX, 8], [1, NPIX]]),
                in_=cB[:])
            for prev in cscr_state[slot]:
                add_dep_helper(wdma.ins, prev.ins, reason="cscr read->write")
            rdma = nc.sync.dma_start(
                out=_ap(dst_padded[:], 2, [[PADX, 128], [1, W]]),
                in_=_ap(cscr.ap(), slot * H * W, [[W, 128], [1, W]]))
            add_dep_helper(rdma.ins, wdma.ins, reason="cscr write->read")
            cscr_state[slot] = [rdma]

        # ================= view 0 phase =================
        tab_loads = []
        for k in range(8):
            d0 = nc.sync.dma_start(
                out=tabT[16 * k:16 * k + 16, :],
                in_=_ap(ptab.ap(), 0, [[NE * 2, 16], [1, NE * 2]]))
            tab_loads.append(d0)
        for di in range(DPC):
            depf = wp.tile([128, W], F32, tag="depf")
            nc.sync.dma_start(out=depf[:], in_=dep.ap()[di])
            warped = warp_view(0, di, depf)
            diffT = wB.tile([128, NPIX], F16, tag="diffT")
            nc.vector.tensor_tensor(out=diffT[:], in0=refB[:], in1=warped[:],
                                    op=OP.subtract)
            dw = nc.sync.dma_start(
                out=_ap(dscr.ap(), di * H * W * C, [[NPIX, 128], [1, NPIX]]),
                in_=diffT[:])
            diff_w[di] = dw
            sq = wB.tile([128, NPIX], F16, tag="sq")
            nc.vector.tensor_tensor(out=sq[:], in0=diffT[:], in1=diffT[:],
                                    op=OP.mult)
            cost_block_to_pixel(sq, costp0[di])

        # ================= view 1 phase =================
        # table overwrite waits for view-0 gathers via the tile-tracked
        # SBUF write-after-read hazard on tabT.
        tab_loads = []
        for k in range(8):
            d1 = nc.sync.dma_start(
                out=tabT[16 * k:16 * k + 16, :],
                in_=_ap(ptab.ap(), C * NE * 2,
                        [[NE * 2, 16], [1, NE * 2]]))
            tab_loads.append(d1)
        for di in range(DPC):
            depf = wp.tile([128, W], F32, tag="depf")
            nc.sync.dma_start(out=depf[:], in_=dep.ap()[di])
            nc.vector.tensor_copy(
                out=_ap(depD[:], di, [[W * DPC, 128], [DPC, W], [1, 1]]),
                in_=depf[:])
            warped = warp_view(1, di, depf)
            diffT = wB.tile([128, NPIX], F16, tag="diffT")
            rd = nc.sync.dma_start(
                out=diffT[:],
                in_=_ap(dscr.ap(), di * H * W * C, [[NPIX, 128], [1, NPIX]]))
            add_dep_helper(rd.ins, diff_w[di].ins, reason="dscr write->read")
            nc.vector.tensor_tensor(out=diffT[:], in0=diffT[:],
                                    in1=warped[:], op=OP.subtract)
            sq = wB.tile([128, NPIX], F16, tag="sq")
            nc.vector.tensor_tensor(out=sq[:], in0=diffT[:], in1=diffT[:],
                                    op=OP.mult)
            costp1 = cp.tile([128, PADX], F16, tag="costp1")
            cost_block_to_pixel(sq, costp1)
            # cost = sqrt(min(c0, c1)), padded [y, 132]
            costp = wp.tile([128, PADX], F16, tag="costp")
            nc.vector.memset(costp[:], 0.0)
            cmin = wp.tile([128, W], F16, tag="cmin")
            nc.vector.tensor_tensor(out=cmin[:],
                                    in0=costp0[di][:, 2:2 + W],
                                    in1=costp1[:, 2:2 + W], op=OP.min)
            nc.scalar.activation(out=costp[:, 2:2 + W], in_=cmin[:],
                                 func=AF.Sqrt)
            if debug:
                cdbg = wp.tile([128, W], F32, tag="cdbg")
                nc.scalar.copy(out=cdbg[:], in_=costp[:, 2:2 + W])
                nc.sync.dma_start(out=dbg_cost.ap()[di], in_=cdbg[:])
            costSC = {}
            for ty in (0, 1, 3, 4):
                t = wp.tile([128, PADX], F16, tag=f"costSC{ty}")
                k = ty - 2
                nc.vector.memset(t[:], 0.0)
                if k < 0:
                    nc.sync.dma_start(out=t[-k:128, :],
                                      in_=costp[0:128 + k, :])
                else:
                    nc.sync.dma_start(out=t[0:128 - k, :],
                                      in_=costp[k:128, :])
                costSC[ty] = t
            costSC[2] = costp
            # depth pads + shifts
            depp = wp.tile([128, PADX], F16, tag="depp")
            nc.vector.memset(depp[:], 0.0)
            nc.scalar.copy(out=depp[:, 2:2 + W], in_=depf[:])
            depSC = {}
            for ty in (0, 1, 3, 4):
                t = wp.tile([128, PADX], F16, tag=f"depSC{ty}")
                k = ty - 2
                nc.vector.memset(t[:], 0.0)
                if k < 0:
                    nc.sync.dma_start(out=t[-k:128, :],
                                      in_=depp[0:128 + k, :])
                else:
                    nc.sync.dma_start(out=t[0:128 - k, :],
                                      in_=depp[k:128, :])
                depSC[ty] = t
            depSC[2] = depp
            # aggregation: num/den over 25 taps
            num = wp.tile([128, W], F32, tag="num")
            den = wp.tile([128, W], F32, tag="den")
            for ty in range(5):
                dvol = wp.tile([128, W * 5], F16, tag="dvol")
                vvv = [[W * 5, 128], [5, W], [1, 5]]
                nc.vector.tensor_tensor(
                    out=_ap(dvol[:], 0, vvv),
                    in0=_ap(depSC[ty][:], 0, [[PADX, 128], [1, W], [1, 5]]),
                    in1=_ap(depp[:], 2, [[PADX, 128], [1, W], [0, 5]]),
                    op=OP.subtract)
                nc.scalar.activation(out=dvol[:], in_=dvol[:], func=AF.Abs)
                evol = wp.tile([128, W * 5], F16, tag="evol")
                nc.scalar.activation(out=evol[:], in_=dvol[:], func=AF.Exp,
                                     bias=tv(7))
                uvol = wp.tile([128, W * 5], F16, tag="uvol")
                nc.vector.tensor_tensor(
                    out=_ap(uvol[:], 0, vvv), in0=_ap(evol[:], 0, vvv),
                    in1=_ap(wfvol[:], ty * 5,
                            [[W * 25, 128], [25, W], [1, 5]]),
                    op=OP.mult)
                nvol = wp.tile([128, W * 5], F16, tag="nvol")
                nc.vector.tensor_tensor(
                    out=_ap(nvol[:], 0, vvv), in0=_ap(uvol[:], 0, vvv),
                    in1=_ap(costSC[ty][:], 0, [[PADX, 128], [1, W], [1, 5]]),
                    op=OP.mult)
                sty = wp.tile([128, W], F16, tag="sty")
                nc.vector.tensor_reduce(out=sty[:], in_=_ap(nvol[:], 0, vvv),
                                        axis=AX.X, op=OP.add)
                ety = wp.tile([128, W], F16, tag="ety")
                nc.vector.tensor_reduce(out=ety[:], in_=_ap(evol[:], 0, vvv),
                                        axis=AX.X, op=OP.add)
                if ty == 0:
                    nc.vector.tensor_copy(out=num[:], in_=sty[:])
                    nc.vector.tensor_copy(out=den[:], in_=ety[:])
                else:
                    nc.vector.tensor_tensor(out=num[:], in0=num[:],
                                            in1=sty[:], op=OP.add)
                    nc.vector.tensor_tensor(out=den[:], in0=den[:],
                                            in1=ety[:], op=OP.add)
            rden = wp.tile([128, W], F32, tag="rden")
            nc.vector.reciprocal(out=rden[:], in_=den[:])
            agg_ap = _ap(aggT[:], di, [[W * DPC, 128], [DPC, W], [1, 1]])
            nc.vector.tensor_tensor(out=agg_ap, in0=num[:], in1=rden[:],
                                    op=OP.mult)
            if debug:
                adbg = wp.tile([128, W], F32, tag="adbg")
                nc.vector.tensor_tensor(out=adbg[:], in0=num[:], in1=rden[:],
                                        op=OP.mult)
                nc.sync.dma_start(out=dbg_agg.ap()[di], in_=adbg[:])

        # ---------------- per-core softmax partials ----------------
        def aggap(di):
            return _ap(aggT[:], di, [[W * DPC, 128], [DPC, W], [1, 1]])

        def depap(di):
            return _ap(depD[:], di, [[W * DPC, 128], [DPC, W], [1, 1]])

        m = pp.tile([128, W], F32, tag="m")
        nc.vector.tensor_tensor(out=m[:], in0=aggap(0), in1=aggap(1),
                                op=OP.max)
        for di in range(2, DPC):
            nc.vector.tensor_tensor(out=m[:], in0=m[:], in1=aggap(di),
                                    op=OP.max)
        s0 = pp.tile([128, W], F32, tag="s0")
        s1 = pp.tile([128, W], F32, tag="s1")
        for di in range(DPC):
            t = wp.tile([128, W], F32, tag="et")
            nc.vector.tensor_tensor(out=t[:], in0=aggap(di), in1=m[:],
                                    op=OP.subtract)
            e = wp.tile([128, W], F32, tag="ee")
            nc.scalar.activation(out=e[:], in_=t[:], func=AF.Exp)
            t1 = wp.tile([128, W], F32, tag="t1")
            nc.vector.tensor_tensor(out=t1[:], in0=e[:], in1=depap(di),
                                    op=OP.mult)
            if di == 0:
                nc.vector.tensor_copy(out=s0[:], in_=e[:])
                nc.vector.tensor_copy(out=s1[:], in_=t1[:])
            else:
                nc.vector.tensor_tensor(out=s0[:], in0=s0[:], in1=e[:],
                                        op=OP.add)
                nc.vector.tensor_tensor(out=s1[:], in0=s1[:], in1=t1[:],
                                        op=OP.add)
        nc.sync.dma_start(out=out3.ap()[0], in_=m[:])
        nc.sync.dma_start(out=out3.ap()[1], in_=s0[:])
        nc.sync.dma_start(out=out3.ap()[2], in_=s1[:])

        for p in reversed(ctx_pools):
            p.__exit__(None, None, None)

    nc.compile()
    return nc


def host_prep(features, intrinsics, cam_to_world, depth_hypo):
    """Build the 8 per-core input maps."""
    features = np.asarray(features, dtype=np.float32)
    intrinsics = np.asarray(intrinsics, dtype=np.float32)
    cam_to_world = np.asarray(cam_to_world, dtype=np.float32)
    depth_hypo = np.asarray(depth_hypo, dtype=np.float32)

    ys, xs = np.meshgrid(np.arange(H, dtype=np.float32),
                         np.arange(W, dtype=np.float32), indexing="ij")
    bsel = np.zeros((128, 8), np.float16)
    for p in range(128):
        bsel[p, p // 16] = 1.0

    # padded pair tables per (v, b): [C, NE, 2] f16
    ptabs = {}
    for b in range(B):
        for vi in range(1, V):
            img = features[vi, b].astype(np.float16)  # [C, H, W]
            pad = np.zeros((C, PH, PW), np.float16)
            pad[:, 2:2 + H, 2:2 + W] = img
            pair = np.zeros((C, PH, PW, 2), np.float16)
            pair[..., 0] = pad
            flatp = pad.reshape(C, -1)
            pair.reshape(C, -1, 2)[:, :-1, 1] = flatp[:, 1:]
            ptabs[(vi - 1, b)] = pair.reshape(C, NE * 2)

    in_maps = []
    for k in range(NCORES):
        b = k // (NCORES // B)
        dlo = DPC * (k % (NCORES // B))
        amap = np.zeros((NV, 3, H, W), np.float32)
        tvv = np.zeros((8,), np.float32)
        for vi in range(1, V):
            src_w2c = np.linalg.inv(cam_to_world[vi, b])
            ref_w2c = np.linalg.inv(cam_to_world[0, b])
            src_KK = src_w2c.copy()
            src_KK[:3, :3] = intrinsics[vi, b]
            ref_KK = ref_w2c.copy()
            ref_KK[:3, :3] = intrinsics[0, b]
            proj = (src_KK @ src_w2c) @ np.linalg.inv(ref_KK @ ref_w2c)
            rot, trans = proj[:3, :3], proj[:3, 3]
            A = (rot[:, 0:1, None] * xs[None] + rot[:, 1:2, None] * ys[None]
                 + rot[:, 2:3, None])  # [3, H, W]
            v = vi - 1
            amap[v, 0] = A[0] * SCALE
            amap[v, 1] = A[1] * SCALE
            amap[v, 2] = A[2]
            tvv[v * 3 + 0] = trans[0] * SCALE
            tvv[v * 3 + 1] = trans[1] * SCALE
            tvv[v * 3 + 2] = trans[2]
        tvv[6] = BIG - 0.5
        tvv[7] = ESHIFT
        ptab = np.stack([ptabs[(0, b)], ptabs[(1, b)]])  # [NV, C, NE*2]
        ref16 = features[0, b].astype(np.float16)        # [C, H, W]
        # refB[16k+c, x*16+y_l] = ref16[c, 16k+y_l, x]
        refB = np.ascontiguousarray(
            ref16.reshape(C, 8, 16, W).transpose(1, 0, 3, 2).reshape(
                128, NPIX))
        in_maps.append({
            "ptab": np.ascontiguousarray(ptab.reshape(NV, C * NE * 2)),
            "refF": np.ascontiguousarray(ref16),
            "refBin": refB,
            "ident": np.eye(128, dtype=np.float16),
            "dep": np.ascontiguousarray(depth_hypo[b, dlo:dlo + DPC]),
            "amap": np.ascontiguousarray(amap),
            "tvec": np.tile(tvv[None, :], (128, 1)).astype(np.float32),
            "bsel": bsel,
        })
    return in_maps


def host_combine(results):
    """Merge per-core softmax partials (m, s0, s1) into [B, H, W]."""
    out = np.zeros((B, H, W), np.float32)
    per_b = NCORES // B
    for b in range(B):
        parts = [np.asarray(results[b * per_b + j]["out3"])
                 for j in range(per_b)]
        ms = np.stack([p[0] for p in parts])
        M = ms.max(axis=0)
        S0 = np.zeros((H, W), np.float64)
        S1 = np.zeros((H, W), np.float64)
        for p in parts:
            w = np.exp(p[0] - M)
            S0 += w * p[1]
            S1 += w * p[2]
        out[b] = (S1 / S0).astype(np.float32)
    return out


def _run_sim(nc, in_maps, outs=("out3",)):
    from concourse.bass_interp import CoreSim
    results = []
    for core in range(NCORES):
        sim = CoreSim(nc, trace=False)
        for k, v in in_maps[core].items():
            sim.tensor(k)[:] = v
        sim.simulate()
        results.append({o: np.array(sim.tensor(o)) for o in outs})
    return results


def _make_runner(nc):
    """Persistent jitted PJRT runner (mirrors bass2jax.run_bass_via_pjrt but
    caches the jitted callable across kernel() calls)."""
    import jax
    from jax.sharding import Mesh, PartitionSpec
    from jax.experimental.shard_map import shard_map
    from concourse.bass2jax import (
        _bass_exec_p, install_neuronx_cc_hook, partition_id_tensor)

    install_neuronx_cc_hook()
    if nc.dbg_addr is not None:
        raise RuntimeError("debug build not supported by cached runner")
    partition_name = (nc.partition_id_tensor.name
                      if nc.partition_id_tensor else None)
    in_names, out_names, out_avals = [], [], []
    for alloc in nc.m.functions[0].allocations:
        if not isinstance(alloc, mybir.MemoryLocationSet):
            continue
        name = alloc.memorylocations[0].name
        if alloc.kind == "ExternalInput":
            if name != partition_name:
                in_names.append(name)
        elif alloc.kind == "ExternalOutput":
            out_names.append(name)
            out_avals.append(jax.core.ShapedArray(
                tuple(alloc.tensor_shape), mybir.dt.np(alloc.dtype)))
    n_params = len(in_names)
    n_outs = len(out_avals)
    all_names = list(in_names) + list(out_names)
    if partition_name is not None:
        all_names.append(partition_name)
    donate = tuple(range(n_params, n_params + n_outs))

    def _body(*args):
        operands = list(args)
        if partition_name is not None:
            operands.append(partition_id_tensor())
        return tuple(_bass_exec_p.bind(
            *operands, out_avals=tuple(out_avals), in_names=tuple(all_names),
            out_names=tuple(out_names), lowering_input_output_aliases=(),
            sim_require_finite=True, sim_require_nnan=True, nc=nc))

    devices = jax.devices()[:NCORES]
    mesh = Mesh(np.asarray(devices), ("core",))
    in_specs = (PartitionSpec("core"),) * (n_params + n_outs)
    out_specs = (PartitionSpec("core"),) * n_outs
    sharded = jax.jit(
        shard_map(_body, mesh=mesh, in_specs=in_specs, out_specs=out_specs,
                  check_rep=False),
        donate_argnums=donate, keep_unused=True)

    def run(in_maps):
        concat_in = [
            np.concatenate([np.asarray(in_maps[c][name])
                            for c in range(NCORES)], axis=0)
            for name in in_names]
        concat_zeros = [
            np.zeros((NCORES * a.shape[0], *a.shape[1:]), a.dtype)
            for a in out_avals]
        out_arrs = sharded(*concat_in, *concat_zeros)
        return [
            {name: np.asarray(out_arrs[i]).reshape(
                NCORES, *out_avals[i].shape)[c]
             for i, name in enumerate(out_names)}
            for c in range(NCORES)]

    return run


def kernel(**inputs):
    if "nc" not in _cached:
        _cached["nc"] = build_program()
    nc = _cached["nc"]
    in_maps = host_prep(**inputs)
    if not _cached.get("runner_broken"):
        try:
            if "runner" not in _cached:
                _cached["runner"] = _make_runner(nc)
            return host_combine(_cached["runner"](in_maps))
        except Exception:
            _cached["runner_broken"] = True
    if _cached.get("hw_broken"):
        return host_combine(_run_sim(nc, in_maps))
    try:
        res = run_bass_kernel_spmd(nc, in_maps, core_ids=list(range(NCORES)))
        return host_combine(res.results)
    except Exception:
        _cached["hw_broken"] = True
        return host_combine(_run_sim(nc, in_maps))


if __name__ == "__main__":
    import reference
    inp = reference.setup_inputs()
    inp = {k: np.asarray(v) for k, v in inp.items()}
    out = kernel(**inp)
    print("kernel out", out.shape, out.dtype)
